# revision 10
# baseline (speedup 1.0000x reference)
"""Trainium2 Bass kernel for nn_CoKT (dual GRU + cross/causal attention + fused linear).

Self-contained: builds an 8-core SPMD Tile kernel, shards tokens (B*S) across
cores (2 batches/core), replicates weights, runs via run_bass_kernel_spmd,
reassembles the full [1024, 256] fp32 output.

Per-core design (128 own tokens, core-local order (bl, s)):
- inter GRU: 768 seqs x 24 steps, seqs GLOBALLY SORTED by length (descending)
  on host; step t only processes the active prefix W(t) = max-over-cores
  active count rounded to 32 (z-freeze +BIG trick protects the padded tail of
  each boundary tile). Cuts seq-step work ~1.8x vs dense. h updated in place.
- attention runs in the sorted frame: q is permuted per 128-seq tile with
  host-built permutation matmuls; softmax normalization and the output
  accumulation go back to token frame via indicator matmuls.
- k/v projections of finished seq-tiles and the intra-attention score/softmax
  chains are interleaved INTO the scan (they only depend on frozen h columns
  resp. xintra), shrinking the serial tail.
- intra GRU: batch 16 x 64 steps, replicated on every core (weight-load bound
  either way); host rotates batches so own 2 batches are columns 0..1.
- xinter/ind fully SBUF-resident (one DMA each); attention weights packed
  into one blob DMA'd mid-scan.
"""
import sys
if "/opt/trn_rl_repo" not in sys.path:
    sys.path.insert(0, "/opt/trn_rl_repo")

import numpy as np
import ml_dtypes

import concourse.bacc as bacc
import concourse.mybir as mybir
import concourse.tile as tile
from concourse.tile import add_dep_helper
from concourse.bass_utils import run_bass_kernel_spmd

F32 = mybir.dt.float32
BF16 = mybir.dt.bfloat16
AF = mybir.ActivationFunctionType
ALU = mybir.AluOpType
AX = mybir.AxisListType

B, S, R, L, D, H = 16, 64, 6, 24, 128, 256
NCORES = 8
BPC = B // NCORES            # 2 batches per core
NTOK = S * BPC               # 128 own tokens
NSEQ = NTOK * R              # 768 inter sequences per core
NT = 256                     # inter token-tile width
BIG = 30000.0
NST = NSEQ // 128            # 6 seq-tiles of 128 in the attention phase

bfc = lambda x: np.ascontiguousarray(np.asarray(x, np.float32).astype(ml_dtypes.bfloat16))
f32c = lambda x: np.ascontiguousarray(np.asarray(x, np.float32))

# attention-weight blob layout: name -> (col offset, width); all 128 rows
# (127-row tensors sit in rows 0:127, cmask in rows 0:64).
_BLOB_NAMES = [
    ("iqw0", 256), ("iqw1", 256), ("ikw0", 256), ("ikw1", 256),
    ("ivw0", 256), ("ivw1", 256), ("ivwx", 256), ("avw0", 256), ("avw1", 256),
    ("AiT0", 256), ("AiT1", 256), ("AaT0", 256), ("AaT1", 256),
    ("LhT0", 256), ("LhT1", 256),
    ("iqwx", 256), ("ikwx", 256), ("aqw", 256), ("akw", 256), ("LxT", 256),
    ("id128", 128), ("Pq", NSEQ), ("Pi", NSEQ), ("cmask", S),
]
_BLOB_OFF = {}
_off = 0
for _nm, _w in _BLOB_NAMES:
    _BLOB_OFF[_nm] = (_off, _w)
    _off += _w
BLOBW = _off
_ROWS127 = {"iqwx", "ikwx", "aqw", "akw", "LxT"}

_BLOBC_NAMES = ["iqb", "ikb", "ivb", "avwx", "avb", "btot"]
_BLOBD_NAMES = ["b_r", "nb_z", "b_in", "b_hn", "aqb", "akb"]


def _tiles_of(w):
    out = []
    o = 0
    while o < w:
        out.append((o, min(NT, w - o)))
        o += NT
    return out


# ----------------------------------------------------------------------------
# device program
# ----------------------------------------------------------------------------

def _coloc(insts):
    first = insts[0]
    for x in insts[1:]:
        add_dep_helper(x.ins, first.ins, sync=True, reason="psum coloc order")


def _after(consumer, last_mm):
    """PSUM banks are single-port: a reader of one co-located half must wait
    until the PE is done with the WHOLE bank (fatal collision otherwise)."""
    add_dep_helper(consumer.ins, last_mm.ins, sync=True, reason="bank read-after-all-mm")


def _emit(nc, tc, di, d_out, W, OFF, MINACT):
    import os
    KLEVEL = int(os.environ.get("KLEVEL", "3"))
    import contextlib
    ctx = contextlib.ExitStack()
    TOTAL = OFF[-1] + W[-1]
    with ctx:
        singles = ctx.enter_context(tc.tile_pool(name="singles", bufs=1))
        sb2 = ctx.enter_context(tc.tile_pool(name="work2", bufs=2))
        sb3 = ctx.enter_context(tc.tile_pool(name="work3", bufs=3))

        def load(name):
            d = di[name]
            t = singles.tile(list(d.shape), d.dtype, tag=name)
            nc.sync.dma_start(out=t, in_=d.ap())
            return t

        # early loads (scan-phase inputs)
        wihT = load("wihT")
        xintra = load("xintra")
        whhT = [load("whh0T"), load("whh1T")]
        blobD = load("blobD")
        xin = load("xinter")
        ind_all = load("indr")
        rT = load("rT")
        xlast = load("xlast")
        bD = {nm: blobD[:, 2 * i:2 * i + 2] for i, nm in enumerate(_BLOBD_NAMES)}
        b_r, nb_z, b_in, b_hn = bD["b_r"], bD["nb_z"], bD["b_in"], bD["b_hn"]
        aqb, akb = bD["aqb"], bD["akb"]

        # deferred loads (attention weights; DMA emitted here, lands mid-scan
        # behind the big xinter transfer on the sync queue)
        blobB = singles.tile([128, BLOBW], BF16, tag="blobB")
        nc.sync.dma_start(out=blobB, in_=di["blobB"].ap())
        blobC = singles.tile([1, 256 * len(_BLOBC_NAMES)], BF16, tag="blobC")
        nc.sync.dma_start(out=blobC, in_=di["blobC"].ap())
        W_ = {}
        for nm, (o_, w_) in _BLOB_OFF.items():
            rows = 127 if nm in _ROWS127 else (S if nm == "cmask" else 128)
            W_[nm] = blobB[0:rows, o_:o_ + w_]
        for i, nm in enumerate(_BLOBC_NAMES):
            W_[nm] = blobC[:, 256 * i:256 * (i + 1)]

        ones = singles.tile([1, 128], BF16, tag="ones")
        nc.vector.memset(ones, 1.0)

        xn_all = singles.tile([128, 2, TOTAL], BF16, tag="xn_all")
        xn_intra = singles.tile([128, 2, B, S], BF16, tag="xn_intra")
        hT_all = singles.tile([128, 2, B, S], BF16, tag="hT_all")
        zeros16 = singles.tile([128, 2, B], BF16, tag="zeros16")
        nc.vector.memset(zeros16, 0.0)
        h_inter = singles.tile([128, 2, NSEQ], BF16, tag="h_inter")
        nc.vector.memset(h_inter, 0.0)

        # GRU-phase psum pools: rz/zz/nn x2 (6) + ia+ib merged (1) + psx (1)
        gru_ps = tc.tile_pool(name="psg", bufs=2, space="PSUM")
        psg = gru_ps.__enter__()
        gru_psi = tc.tile_pool(name="psi", bufs=1, space="PSUM")
        psi = gru_psi.__enter__()
        gru_psx = tc.tile_pool(name="psx", bufs=1, space="PSUM")
        psx = gru_psx.__enter__()

        # ---------------- phase 1 pieces: xn = w_ih_n @ x (+b_in via evac) ----
        def xn_inter_step(t):
            for (o, w) in _tiles_of(W[t]):
                xt = xin[:, OFF[t] + o: OFF[t] + o + w]
                px = psg.tile([128, 2, NT], F32, tag="rz")
                m0 = nc.tensor.matmul(px[:, 0, :w], wihT[:, 512:640],
                                      xt, start=True, stop=False)
                m1 = nc.tensor.matmul(px[:, 1, :w], wihT[:, 640:768],
                                      xt, start=False, stop=True)
                _coloc([m0, m1])
                dst = xn_all[:, :, OFF[t] + o: OFF[t] + o + w]
                ev0 = nc.scalar.activation(dst[:, 0, :], px[:, 0, :w], AF.Identity,
                                           bias=b_in[:, 0:1])
                _after(ev0, m1)
                nc.vector.tensor_scalar_add(dst[:, 1, :], px[:, 1, :w], b_in[:, 1:2])

        def xn_intra_all():
            xflat = xintra.rearrange("d b s -> d (b s)")
            for j in range(2):
                o = j * 512
                for ci in range(2):
                    px = psg.tile([128, 512], F32, tag="nn")
                    nc.tensor.matmul(px, wihT[:, 512 + ci * 128: 640 + ci * 128],
                                     xflat[:, o:o + 512], start=True, stop=True)
                    dst = xn_intra.rearrange("p c b s -> p c (b s)")[:, ci, o:o + 512]
                    if ci == 0:
                        nc.scalar.activation(dst, px, AF.Identity, bias=b_in[:, 0:1])
                    else:
                        nc.vector.tensor_scalar_add(dst, px, b_in[:, 1:2])

        # ---------------- phase 2: scans ----------------
        def inter_tile(t, o, w):
            h = h_inter
            freeze = (o + w) > MINACT[t]
            rz = psg.tile([128, 2, NT], F32, tag="rz")
            zz = psg.tile([128, 2, NT], F32, tag="zz")
            nn = psg.tile([128, 2, NT], F32, tag="nn")
            xt = xin[:, OFF[t] + o: OFF[t] + o + w]

            def gate_bank(ps, g0, fz):
                insts = []
                last = None
                for ci in range(2):
                    g = g0 + ci
                    sl = slice(g * 128, (g + 1) * 128)
                    mm = nc.tensor.matmul(ps[:, ci, :w], wihT[:, sl], xt,
                                          start=(ci == 0), stop=False)
                    insts.append(mm)
                    nc.tensor.matmul(ps[:, ci, :w], whhT[0][:, sl], h[:, 0, o:o + w],
                                     start=False, stop=False)
                    last = nc.tensor.matmul(ps[:, ci, :w], whhT[1][:, sl],
                                            h[:, 1, o:o + w],
                                            start=False, stop=(not fz) and ci == 1)
                    if fz:
                        last = nc.tensor.matmul(
                            ps[:, ci, :w], ones,
                            ind_all[:, OFF[t] + o: OFF[t] + o + w],
                            start=False, stop=(ci == 1))
                _coloc(insts)
                return last

            rz_last = gate_bank(rz, 0, False)
            zz_last = gate_bank(zz, 2, freeze)
            i0 = nc.tensor.matmul(nn[:, 0, :w], whhT[0][:, 512:640], h[:, 0, o:o + w],
                                  start=True, stop=False)
            nc.tensor.matmul(nn[:, 0, :w], whhT[1][:, 512:640], h[:, 1, o:o + w],
                             start=False, stop=False)
            i1 = nc.tensor.matmul(nn[:, 1, :w], whhT[0][:, 640:768], h[:, 0, o:o + w],
                                  start=False, stop=False)
            nn_last = nc.tensor.matmul(nn[:, 1, :w], whhT[1][:, 640:768],
                                       h[:, 1, o:o + w], start=False, stop=True)
            _coloc([i0, i1])

            r_sb = sb3.tile([128, 2, NT], BF16, tag="r_sb")
            zc_sb = sb3.tile([128, 2, NT], BF16, tag="zc_sb")
            t1_sb = sb3.tile([128, 2, NT], BF16, tag="t1_sb")
            u_sb = sb3.tile([128, 2, NT], BF16, tag="u_sb")
            n_sb = sb3.tile([128, 2, NT], BF16, tag="n_sb")
            d_sb = sb3.tile([128, 2, NT], BF16, tag="d_sb")
            f_sb = sb3.tile([128, 2, NT], BF16, tag="f_sb")
            for ci in range(2):
                _after(nc.scalar.activation(r_sb[:, ci, :w], rz[:, ci, :w], AF.Sigmoid,
                                            bias=b_r[:, ci:ci + 1]), rz_last)
                _after(nc.scalar.activation(zc_sb[:, ci, :w], zz[:, ci, :w], AF.Sigmoid,
                                            bias=nb_z[:, ci:ci + 1], scale=-1.0),
                       zz_last)
                _after(nc.vector.scalar_tensor_tensor(
                    t1_sb[:, ci, :w], nn[:, ci, :w], b_hn[:, ci:ci + 1], r_sb[:, ci, :w],
                    op0=ALU.add, op1=ALU.mult), nn_last)
            nc.vector.tensor_add(u_sb[:, :, :w], t1_sb[:, :, :w],
                                 xn_all[:, :, OFF[t] + o: OFF[t] + o + w])
            nc.scalar.activation(n_sb[:, :, :w], u_sb[:, :, :w], AF.Tanh)
            hsl = h[:, :, o:o + w]
            nc.gpsimd.tensor_sub(d_sb[:, :, :w], hsl, n_sb[:, :, :w])
            nc.gpsimd.tensor_mul(f_sb[:, :, :w], zc_sb[:, :, :w], d_sb[:, :, :w])
            nc.vector.tensor_sub(hsl, hsl, f_sb[:, :, :w])

        def intra_step(s):
            hprev = zeros16 if s == 0 else hT_all[:, :, :, s - 1]
            iab = psi.tile([128, 6, B], F32, tag="iab")
            ia = iab[:, 0:4, :]
            ib = iab[:, 4:6, :]
            xt = xintra[:, :, s]
            insts = []
            for g in range(4):
                sl = slice(g * 128, (g + 1) * 128)
                mm = nc.tensor.matmul(ia[:, g, :], wihT[:, sl], xt,
                                      start=(g == 0), stop=False)
                insts.append(mm)
                nc.tensor.matmul(ia[:, g, :], whhT[0][:, sl], hprev[:, 0, :],
                                 start=False, stop=False)
                nc.tensor.matmul(ia[:, g, :], whhT[1][:, sl], hprev[:, 1, :],
                                 start=False, stop=False)
            ib_last = None
            for ci in range(2):
                sl = slice(512 + ci * 128, 512 + (ci + 1) * 128)
                mm = nc.tensor.matmul(ib[:, ci, :], whhT[0][:, sl], hprev[:, 0, :],
                                      start=False, stop=False)
                insts.append(mm)
                ib_last = nc.tensor.matmul(ib[:, ci, :], whhT[1][:, sl], hprev[:, 1, :],
                                           start=False, stop=(ci == 1))
            _coloc(insts)

            r_sb = sb2.tile([128, 2, B], BF16, tag="ir_sb")
            zc_sb = sb2.tile([128, 2, B], BF16, tag="izc_sb")
            t1_sb = sb2.tile([128, 2, B], BF16, tag="it1_sb")
            u_sb = sb2.tile([128, 2, B], BF16, tag="iu_sb")
            n_sb = sb2.tile([128, 2, B], BF16, tag="in_sb")
            d_sb = sb2.tile([128, 2, B], BF16, tag="id_sb")
            f_sb = sb2.tile([128, 2, B], BF16, tag="if_sb")
            for ci in range(2):
                _after(nc.scalar.activation(r_sb[:, ci, :], ia[:, ci, :], AF.Sigmoid,
                                            bias=b_r[:, ci:ci + 1]), ib_last)
                _after(nc.scalar.activation(zc_sb[:, ci, :], ia[:, 2 + ci, :],
                                            AF.Sigmoid, bias=nb_z[:, ci:ci + 1],
                                            scale=-1.0), ib_last)
                _after(nc.vector.scalar_tensor_tensor(
                    t1_sb[:, ci, :], ib[:, ci, :], b_hn[:, ci:ci + 1], r_sb[:, ci, :],
                    op0=ALU.add, op1=ALU.mult), ib_last)
            nc.vector.tensor_add(u_sb, t1_sb, xn_intra[:, :, :, s])
            nc.scalar.activation(n_sb, u_sb, AF.Tanh)
            nc.gpsimd.tensor_sub(d_sb, hprev, n_sb)
            nc.gpsimd.tensor_mul(f_sb, zc_sb, d_sb)
            nc.vector.tensor_sub(hT_all[:, :, :, s], hprev, f_sb)

        # ---------------- interleaved attention pieces ----------------
        k_sb = singles.tile([128, NST, 256], BF16, tag="k_sb")
        v_sb = singles.tile([128, NST, 256], BF16, tag="v_sb")
        qa_sb = singles.tile([128, 2, 128], BF16, tag="qa_sb")
        ka_sb = singles.tile([128, 2, 128], BF16, tag="ka_sb")
        paT_all = singles.tile([S, BPC, 2, S], BF16, tag="paT_all")

        hflat = hT_all.rearrange("p c b s -> p c (b s)")   # [128, 2, 1024]
        hown = [hflat[:, ci, 0:NTOK] for ci in range(2)]    # [128, 128] each
        xflat_i = xintra.rearrange("d b s -> d (b s)")
        xp_own = xflat_i[0:127, 0:NTOK]                     # [127, 128]

        def proj_psx(lhs_chunks, rhs_tiles, bias_tile, out_sb):
            p = psx.tile([128, 256], F32, tag="x", name="p")
            first = True
            mm = None
            for (lt, rt) in zip(lhs_chunks, rhs_tiles):
                mm = nc.tensor.matmul(p, lt, rt, start=first, stop=False)
                first = False
            mm = nc.tensor.matmul(p, ones, bias_tile, start=False, stop=True)
            nc.scalar.copy(out_sb, p)

        def kv_tile(s_):
            cols = slice(s_ * 128, (s_ + 1) * 128)
            proj_psx([h_inter[:, 0, cols], h_inter[:, 1, cols], rT[0:127, cols]],
                     [W_["ikw0"], W_["ikw1"], W_["ikwx"]], W_["ikb"], k_sb[:, s_, :])
            proj_psx([h_inter[:, 0, cols], h_inter[:, 1, cols], rT[:, cols]],
                     [W_["ivw0"], W_["ivw1"], W_["ivwx"]], W_["ivb"], v_sb[:, s_, :])

        def qaka():
            for wn, ob, bias in (("aqw", qa_sb, aqb), ("akw", ka_sb, akb)):
                ps = psx.tile([128, 2, 128], F32, tag="x", name="ps")
                insts = []
                for ci in range(2):
                    mm = nc.tensor.matmul(ps[:, ci, :],
                                          W_[wn][:, ci * 128:(ci + 1) * 128],
                                          xp_own, start=(ci == 0), stop=(ci == 1))
                    insts.append(mm)
                _coloc(insts)
                for ci in range(2):
                    _after(nc.scalar.activation(ob[:, ci, :], ps[:, ci, :],
                                                AF.Identity, bias=bias[:, ci:ci + 1]),
                           insts[-1])

        def intra_chain(bl, hh):
            sca = psx.tile([S, S], F32, tag="x", name="sca")
            mm = nc.tensor.matmul(sca, qa_sb[:, hh, bl * S:(bl + 1) * S],
                                  ka_sb[:, hh, bl * S:(bl + 1) * S],
                                  start=True, stop=True)
            ms = sb3.tile([S, S], BF16, tag="ms")
            nc.vector.tensor_add(ms, sca, W_["cmask"])
            ex = sb3.tile([S, S], BF16, tag="ex")
            nc.scalar.activation(ex, ms, AF.Exp)
            rs = sb3.tile([S, 1], F32, tag="rs")
            nc.vector.tensor_reduce(rs, ex, axis=AX.X, op=ALU.add)
            ri = sb3.tile([S, 1], F32, tag="ri")
            nc.vector.reciprocal(ri, rs)
            pa = sb3.tile([S, S], BF16, tag="pa")
            nc.vector.tensor_scalar_mul(pa, ex, ri)
            ptp = psx.tile([S, S], BF16, tag="x", name="ptp")
            nc.tensor.transpose(ptp, pa, W_["id128"][0:S, 0:S])
            nc.vector.tensor_copy(paT_all[:, bl, hh, :], ptp)

        # k/v tiles become final when the active width drops below their cols
        kv_after_step = {}
        for s_ in range(NST):
            ready = max((t for t in range(L) if W[t] > s_ * 128), default=-1)
            kv_after_step.setdefault(ready, []).append(s_)

        # ---------------- emission schedule ----------------
        xn_intra_all()
        XN_LEAD = 6
        for t in range(XN_LEAD):
            xn_inter_step(t)

        if KLEVEL == 1:
            ob = sb2.tile([128, 256], F32, tag="out_sb", name="ob")
            nc.vector.tensor_copy(ob, xn_all[:, 0, 0:256])
            nc.sync.dma_start(out=d_out.ap(), in_=ob)
            gru_psx.__exit__(None, None, None)
            gru_psi.__exit__(None, None, None)
            gru_ps.__exit__(None, None, None)
            return

        inter_iters = [(t, o, w) for t in range(L) for (o, w) in _tiles_of(W[t])]
        NT_TOT = len(inter_iters)
        chain_at = {10: (0, 0), 18: (0, 1), 26: (1, 0), 34: (1, 1)}
        emitted = 0
        xn_done = XN_LEAD
        kv_done = set()
        for i in range(S):
            intra_step(i)
            if i == 4:
                qaka()
            if i in chain_at:
                intra_chain(*chain_at[i])
            while xn_done < L and xn_done < XN_LEAD + (i * (L - XN_LEAD)) // 45:
                xn_inter_step(xn_done)
                xn_done += 1
            target = min(NT_TOT, ((i + 1) * NT_TOT) // S)
            while emitted < target:
                t, o, w = inter_iters[emitted]
                assert t < max(xn_done, XN_LEAD)
                inter_tile(t, o, w)
                emitted += 1
                last_of_step = (emitted == NT_TOT or inter_iters[emitted][0] != t)
                if last_of_step:
                    for s_ in kv_after_step.get(t, []):
                        kv_tile(s_)
                        kv_done.add(s_)
        his_last = h_inter
        for s_ in range(NST):
            if s_ not in kv_done:
                kv_tile(s_)
        gru_psx.__exit__(None, None, None)
        gru_psi.__exit__(None, None, None)
        gru_ps.__exit__(None, None, None)

        if KLEVEL == 2:
            ob = sb2.tile([128, 256], F32, tag="out_sb", name="ob")
            nc.vector.tensor_copy(ob[:, 0:128], his_last[:, 0, 0:128])
            nc.vector.tensor_copy(ob[:, 128:256], hflat[:, 0, 0:128])
            nc.sync.dma_start(out=d_out.ap(), in_=ob)
            return

        # ---------------- phase 3: attention + fused final ----------------
        psa = ctx.enter_context(tc.tile_pool(name="psa", bufs=2, space="PSUM"))
        psb = ctx.enter_context(tc.tile_pool(name="psb", bufs=2, space="PSUM"))
        psf = ctx.enter_context(tc.tile_pool(name="psf", bufs=1, space="PSUM"))

        def proj(lhs_chunks, rhs_tiles, bias_tile, m_parts=128):
            p = psa.tile([m_parts, 256], F32, tag="proj")
            first = True
            for (lt, rt) in zip(lhs_chunks, rhs_tiles):
                nc.tensor.matmul(p, lt, rt, start=first, stop=False)
                first = False
            nc.tensor.matmul(p, ones[:, 0:m_parts], bias_tile, start=False, stop=True)
            return p

        q_ps = proj([hown[0], hown[1], xp_own],
                    [W_["iqw0"], W_["iqw1"], W_["iqwx"]], W_["iqb"])
        q_sb = sb2.tile([128, 256], BF16, tag="q_sb")
        nc.scalar.copy(q_sb, q_ps)

        # sorted-frame attention: q permute + scores + exp per 128-seq tile
        e_sb = singles.tile([128, NST, 2], BF16, tag="e_sb")
        e32 = singles.tile([128, NST, 2], F32, tag="e32")
        for s_ in range(NST):
            cols = slice(s_ * 128, (s_ + 1) * 128)
            qpp = psa.tile([128, 256], F32, tag="proj")
            nc.tensor.matmul(qpp, W_["Pq"][:, cols], q_sb, start=True, stop=True)
            qp_sb = sb3.tile([128, 256], BF16, tag="qp_sb")
            nc.scalar.copy(qp_sb, qpp)
            scratch = sb3.tile([128, 2, 128], BF16, tag="ttr_scratch")
            nc.vector.tensor_mul(scratch,
                                 qp_sb.rearrange("p (c n) -> p c n", c=2),
                                 k_sb[:, s_, :].rearrange("p (c n) -> p c n", c=2))
            sc = sb3.tile([128, 2, 1], F32, tag="sc")
            nc.vector.tensor_reduce(sc, scratch, axis=AX.X, op=ALU.add)
            nc.scalar.activation(e32[:, s_, :].rearrange("p (c one) -> p c one", c=2),
                                 sc, AF.Exp)
            nc.vector.tensor_copy(e_sb[:, s_, :], e32[:, s_, :])

        # esum per token + weighted values, co-located in one PSUM bank:
        # esum at cols 256:258, o at cols 0:256; single start=True clears bank.
        acc = psf.tile([128, 512], F32, tag="acc")
        esum_ps = acc[:, 256:258]
        o_ps = acc[:, 0:256]
        acc_insts = []
        for s_ in range(NST):
            cols = slice(s_ * 128, (s_ + 1) * 128)
            acc_insts.append(nc.tensor.matmul(
                esum_ps, W_["Pi"][:, cols], e_sb[:, s_, :],
                start=(s_ == 0), stop=False))
        ow_last = None
        for s_ in range(NST):
            cols = slice(s_ * 128, (s_ + 1) * 128)
            vw = sb3.tile([128, 256], BF16, tag="vw")
            for hh in range(2):
                hs = slice(hh * 128, (hh + 1) * 128)
                nc.vector.tensor_scalar_mul(vw[:, hs], v_sb[:, s_, hs],
                                            e32[:, s_, hh:hh + 1])
            ow_last = nc.tensor.matmul(o_ps, W_["Pi"][:, cols], vw,
                                       start=False, stop=(s_ == NST - 1))
            acc_insts.append(ow_last)
        _coloc(acc_insts)
        einv = sb2.tile([128, 2], F32, tag="einv")
        _after(nc.vector.reciprocal(einv, esum_ps), ow_last)
        o_i = sb2.tile([128, 256], BF16, tag="o_i")
        for hh in range(2):
            hs = slice(hh * 128, (hh + 1) * 128)
            _after(nc.vector.tensor_scalar_mul(o_i[:, hs], o_ps[:, hs],
                                               einv[:, hh:hh + 1]), ow_last)

        if KLEVEL == 27:
            ob = sb2.tile([128, 256], F32, tag="out_sb", name="ob")
            nc.vector.tensor_copy(ob, o_i)
            nc.sync.dma_start(out=d_out.ap(), in_=ob)
            return

        oiT = sb2.tile([128, 2, 128], BF16, tag="oiT")
        for ci in range(2):
            tp = psb.tile([128, 128], BF16, tag="tp", name="tp")
            nc.tensor.transpose(tp, o_i[:, ci * 128:(ci + 1) * 128], W_["id128"])
            nc.vector.tensor_copy(oiT[:, ci, :], tp)

        va_sb = []
        for bl in range(BPC):
            vp = proj([hT_all[:, 0, bl, :], hT_all[:, 1, bl, :], xlast[:, bl, :]],
                      [W_["avw0"], W_["avw1"], W_["avwx"]], W_["avb"], m_parts=S)
            vb = sb2.tile([S, 256], BF16, tag="va_sb")
            nc.scalar.copy(vb, vp)
            va_sb.append(vb)

        oaT = sb2.tile([128, 2, 128], BF16, tag="oaT")
        for bl in range(BPC):
            for hh in range(2):
                op = psb.tile([128, S], F32, tag="tp")
                nc.tensor.matmul(op, va_sb[bl][:, hh * 128:(hh + 1) * 128],
                                 paT_all[:, bl, hh, :], start=True, stop=True)
                nc.vector.tensor_copy(oaT[:, hh, bl * S:(bl + 1) * S], op)

        if KLEVEL == 29:
            ob = sb2.tile([128, 256], F32, tag="out_sb", name="ob")
            nc.vector.tensor_copy(ob[:, 0:128], oaT[:, 0, :])
            nc.vector.tensor_copy(ob[0:64, 128:256], va_sb[0][:, 0:128])
            nc.sync.dma_start(out=d_out.ap(), in_=ob[:, :])
            return

        # fused final projection
        fo = psf.tile([128, 512], F32, tag="acc", name="fo")[:, 0:256]
        nc.tensor.matmul(fo, oiT[:, 0, :], W_["AiT0"], start=True, stop=False)
        nc.tensor.matmul(fo, oiT[:, 1, :], W_["AiT1"], start=False, stop=False)
        nc.tensor.matmul(fo, oaT[:, 0, :], W_["AaT0"], start=False, stop=False)
        nc.tensor.matmul(fo, oaT[:, 1, :], W_["AaT1"], start=False, stop=False)
        nc.tensor.matmul(fo, hown[0], W_["LhT0"], start=False, stop=False)
        nc.tensor.matmul(fo, hown[1], W_["LhT1"], start=False, stop=False)
        nc.tensor.matmul(fo, xp_own, W_["LxT"], start=False, stop=False)
        nc.tensor.matmul(fo, ones, W_["btot"], start=False, stop=True)
        out_sb = sb2.tile([128, 256], F32, tag="out_sb")
        nc.vector.tensor_copy(out_sb, fo)
        nc.sync.dma_start(out=d_out.ap(), in_=out_sb)


def _build(W, MINACT):
    OFF = [0]
    for t in range(L - 1):
        OFF.append(OFF[t] + W[t])
    TOTAL = OFF[-1] + W[-1]

    nc = bacc.Bacc("TRN2", target_bir_lowering=False, debug=False)
    di = {}

    def inp(name, shape, dt=BF16):
        di[name] = nc.dram_tensor(name, list(shape), dt, kind="ExternalInput")

    inp("xinter", [128, TOTAL])
    inp("xintra", [128, B, S])
    inp("xlast", [1, B, S])
    inp("rT", [128, NSEQ])
    inp("indr", [1, TOTAL])
    inp("wihT", [128, 768])
    inp("whh0T", [128, 768])
    inp("whh1T", [128, 768])
    inp("blobB", [128, BLOBW])
    inp("blobC", [1, 256 * len(_BLOBC_NAMES)])
    inp("blobD", [128, 2 * len(_BLOBD_NAMES)], F32)

    d_out = nc.dram_tensor("out", [NTOK, 256], F32, kind="ExternalOutput")

    with tile.TileContext(nc) as tc:
        _emit(nc, tc, di, d_out, W, OFF, MINACT)
    nc.compile()
    return nc


# ----------------------------------------------------------------------------
# host-side prep
# ----------------------------------------------------------------------------

def _plan(inter_len):
    """Per-core length-sort plan + shared compile-time widths."""
    lens5 = np.asarray(inter_len, np.int64).reshape(B, S, R)
    orders, lens_sorted = [], []
    act = np.zeros((NCORES, L), np.int64)
    for c in range(NCORES):
        lens = lens5[[2 * c, 2 * c + 1]].reshape(NSEQ)
        order = np.argsort(-lens, kind="stable")
        ls = lens[order]
        orders.append(order)
        lens_sorted.append(ls)
        for t in range(L):
            act[c, t] = int((ls > t).sum())
    W = [min(NSEQ, int(-32 * (-(act[:, t].max()) // 32))) for t in range(L)]
    W = [max(32, w) for w in W]
    for t in range(1, L):
        W[t] = min(W[t], W[t - 1])
    MINACT = [int(act[:, t].min()) for t in range(L)]
    OFF = [0]
    for t in range(L - 1):
        OFF.append(OFF[t] + W[t])
    return orders, lens_sorted, W, MINACT, OFF


def prep_in_maps(inputs):
    inp = {k: np.asarray(v) for k, v in inputs.items()}
    w_ih = f32c(inp["w_ih"])
    w_hh = f32c(inp["w_hh"])
    b_ih = f32c(inp["b_ih"])
    b_hh = f32c(inp["b_hh"])
    b_rz = b_ih[:2 * H] + b_hh[:2 * H]
    sq = np.sqrt(128.0)

    e = np.exp(f32c(inp["wr"])[0, 0] - f32c(inp["wr"])[0, 0].max())
    w01 = e / e.sum()
    ln_w = f32c(inp["ln_w"])
    L_v, L_h, L_x = ln_w[:, :H], ln_w[:, H:2 * H], ln_w[:, 2 * H:]
    Ai = w01[0] * (L_v @ f32c(inp["io_w"]))
    Aa = w01[1] * (L_v @ f32c(inp["ao_w"]))
    btot = f32c(inp["ln_b"]) + L_v @ (w01[0] * f32c(inp["io_b"]) + w01[1] * f32c(inp["ao_b"]))

    iq_w = f32c(inp["iq_w"]) / sq
    iq_b = f32c(inp["iq_b"]) / sq
    aq_w = f32c(inp["aq_w"]) / sq
    aq_b = f32c(inp["aq_b"]) / sq

    def chunks2(m):  # [128,2] fp32 per-partition chunk tiles
        return f32c(np.stack([m[:128], m[128:256]], axis=1))

    orders, lens_sorted, W, MINACT, OFF = _plan(inp["inter_len"])
    TOTAL = OFF[-1] + W[-1]

    x_bs = f32c(inp["intra_x"])                     # [B,S,D]
    his5 = f32c(inp["inter_his"]).reshape(B, S, R, L, D)
    r5 = f32c(inp["inter_r"]).reshape(B, S, R, D)

    # shared weight blobs
    bw = {
        "iqw0": iq_w.T[0:128], "iqw1": iq_w.T[128:256], "iqwx": iq_w.T[256:383],
        "ikw0": inp["ik_w"].T[0:128], "ikw1": inp["ik_w"].T[128:256],
        "ikwx": inp["ik_w"].T[256:383],
        "ivw0": inp["iv_w"].T[0:128], "ivw1": inp["iv_w"].T[128:256],
        "ivwx": inp["iv_w"].T[256:384],
        "aqw": aq_w.T, "akw": f32c(inp["ak_w"]).T,
        "avw0": inp["av_w"].T[0:128], "avw1": inp["av_w"].T[128:256],
        "AiT0": Ai.T[0:128], "AiT1": Ai.T[128:256],
        "AaT0": Aa.T[0:128], "AaT1": Aa.T[128:256],
        "LhT0": L_h.T[0:128], "LhT1": L_h.T[128:256], "LxT": L_x.T,
        "id128": np.eye(128, dtype=np.float32),
        "cmask": np.where(np.tril(np.ones((S, S), bool)), 0.0, -BIG),
    }
    blobC = np.zeros((1, 256 * len(_BLOBC_NAMES)), np.float32)
    bc = {
        "iqb": iq_b, "ikb": f32c(inp["ik_b"]), "ivb": f32c(inp["iv_b"]),
        "avwx": f32c(inp["av_w"]).T[256], "avb": f32c(inp["av_b"]), "btot": btot,
    }
    for i, nm in enumerate(_BLOBC_NAMES):
        blobC[0, 256 * i:256 * i + len(bc[nm])] = bc[nm]
    blobD = np.zeros((128, 2 * len(_BLOBD_NAMES)), np.float32)
    bd = {
        "b_r": chunks2(b_rz[:H]), "nb_z": chunks2(-b_rz[H:]),
        "b_in": chunks2(b_ih[2 * H:]), "b_hn": chunks2(b_hh[2 * H:]),
        "aqb": chunks2(aq_b), "akb": chunks2(f32c(inp["ak_b"])),
    }
    for i, nm in enumerate(_BLOBD_NAMES):
        blobD[:, 2 * i:2 * i + 2] = bd[nm]

    shared = dict(
        wihT=bfc(w_ih.T),
        whh0T=bfc(w_hh.T[0:128]),
        whh1T=bfc(w_hh.T[128:256]),
        blobC=bfc(blobC),
        blobD=f32c(blobD),
    )

    in_maps = []
    for c in range(NCORES):
        bsel = [2 * c, 2 * c + 1]
        order = orders[c]
        ls = lens_sorted[c]
        # inter: sorted seq order; orig col order is ((bl,s),r)
        his_cols = his5[bsel].reshape(NSEQ, L, D)[order]    # [NSEQ, L, D]
        xint = np.zeros((D, TOTAL), np.float32)
        ind = np.zeros((1, TOTAL), np.float32)
        for t in range(L):
            o, w = OFF[t], W[t]
            xint[:, o:o + w] = his_cols[:w, t, :].T
            ind[0, o:o + w] = BIG * (t >= ls[:w])
        rTc = r5[bsel].reshape(NSEQ, D)[order].T            # [D, NSEQ]
        # permutation matrices: tok_of[p] for sorted position p
        tok_of = order // R
        Pq = np.zeros((128, NSEQ), np.float32)
        Pi = np.zeros((128, NSEQ), np.float32)
        for s_ in range(NST):
            for pl in range(128):
                tok = tok_of[s_ * 128 + pl]
                Pq[tok, s_ * 128 + pl] = 1.0
                Pi[pl, s_ * 128 + tok] = 1.0
        blobB = np.zeros((128, BLOBW), np.float32)
        for nm, (o_, w_) in _BLOB_OFF.items():
            src = {"Pq": Pq, "Pi": Pi}.get(nm)
            if src is None:
                src = bw[nm]
            blobB[0:src.shape[0], o_:o_ + src.shape[1]] = src
        # intra: batches rotated so own batches are 0..1; (d, b, s) layout
        rolled = np.roll(x_bs, -2 * c, axis=0)
        xia = rolled.transpose(2, 0, 1)             # [D, B, S]
        m = dict(shared)
        m.update(
            xinter=bfc(xint),
            xintra=bfc(xia),
            xlast=bfc(xia[127:128]),
            rT=bfc(rTc),
            indr=bfc(ind),
            blobB=bfc(blobB),
        )
        in_maps.append(m)
    return in_maps, W, MINACT


def assemble(core_outs):
    o = np.stack([np.asarray(co, np.float32) for co in core_outs])  # [8,128,256]
    return np.ascontiguousarray(o.reshape(B * S, 256))


_CACHE = {}


def kernel(**inputs) -> np.ndarray:
    in_maps, W, MINACT = prep_in_maps(inputs)
    key = (tuple(W), tuple(MINACT))
    if _CACHE.get("key") != key:
        _CACHE["nc"] = _build(W, MINACT)
        _CACHE["key"] = key
    nc = _CACHE["nc"]
    res = run_bass_kernel_spmd(nc, in_maps, core_ids=list(range(NCORES)))
    return assemble([r["out"] for r in res.results])


# revision 18
# speedup vs baseline: 1.1523x; 1.1523x over previous
"""Trainium2 Bass kernel for nn_CoKT (dual GRU + cross/causal attention + fused linear).

Self-contained: builds an 8-core SPMD Tile kernel, shards tokens (B*S) across
cores (2 batches/core), replicates weights, runs via run_bass_kernel_spmd,
reassembles the full [1024, 256] fp32 output.

Per-core design (128 own tokens, core-local order (bl, s)):
- inter GRU: 768 seqs x 24 steps, seqs GLOBALLY SORTED by length (descending)
  on host; step t only processes the active prefix W(t) = max-over-cores
  active count rounded to 32 (z-freeze +BIG trick protects the padded tail of
  each boundary tile). Cuts seq-step work ~1.8x vs dense. h updated in place.
- attention runs in the sorted frame: q is permuted per 128-seq tile with
  host-built permutation matmuls; softmax normalization and the output
  accumulation go back to token frame via indicator matmuls.
- k/v projections of finished seq-tiles and the intra-attention score/softmax
  chains are interleaved INTO the scan (they only depend on frozen h columns
  resp. xintra), shrinking the serial tail.
- intra GRU: batch 16 x 64 steps, replicated on every core (weight-load bound
  either way); host rotates batches so own 2 batches are columns 0..1.
- xinter/ind fully SBUF-resident (one DMA each); attention weights packed
  into one blob DMA'd mid-scan.
"""
import sys
if "/opt/trn_rl_repo" not in sys.path:
    sys.path.insert(0, "/opt/trn_rl_repo")

import numpy as np
import ml_dtypes

import concourse.bacc as bacc
import concourse.mybir as mybir
import concourse.tile as tile
from concourse.tile import add_dep_helper
from concourse.bass_utils import run_bass_kernel_spmd

F32 = mybir.dt.float32
BF16 = mybir.dt.bfloat16
AF = mybir.ActivationFunctionType
ALU = mybir.AluOpType
AX = mybir.AxisListType

B, S, R, L, D, H = 16, 64, 6, 24, 128, 256
NCORES = 8
BPC = B // NCORES            # 2 batches per core
NTOK = S * BPC               # 128 own tokens
NSEQ = NTOK * R              # 768 inter sequences per core
NT = 256                     # inter token-tile width
BIG = 30000.0
NST = NSEQ // 128            # 6 seq-tiles of 128 in the attention phase

bfc = lambda x: np.ascontiguousarray(np.asarray(x, np.float32).astype(ml_dtypes.bfloat16))
f32c = lambda x: np.ascontiguousarray(np.asarray(x, np.float32))

# attention-weight blob layout: name -> (col offset, width); all 128 rows
# (127-row tensors sit in rows 0:127, cmask in rows 0:64).
_BLOB_NAMES = [
    ("iqw0", 256), ("iqw1", 256), ("ikw0", 256), ("ikw1", 256),
    ("ivw0", 256), ("ivw1", 256), ("ivwx", 256), ("avw0", 256), ("avw1", 256),
    ("AiT0", 256), ("AiT1", 256), ("AaT0", 256), ("AaT1", 256),
    ("LhT0", 256), ("LhT1", 256),
    ("iqwx", 256), ("ikwx", 256), ("aqw", 256), ("akw", 256), ("LxT", 256),
    ("id128", 128), ("Pq", NSEQ), ("Pi", NSEQ), ("cmask", S),
]
_BLOB_OFF = {}
_off = 0
for _nm, _w in _BLOB_NAMES:
    _BLOB_OFF[_nm] = (_off, _w)
    _off += _w
BLOBW = _off
_ROWS127 = {"iqwx", "ikwx", "aqw", "akw", "LxT"}

_BLOBC_NAMES = ["iqb", "ikb", "ivb", "avwx", "avb", "btot"]
_BLOBD_NAMES = ["b_r", "nb_z", "b_in", "b_hn", "aqb", "akb"]


def _tiles_of(w):
    out = []
    o = 0
    while o < w:
        out.append((o, min(NT, w - o)))
        o += NT
    return out


# ----------------------------------------------------------------------------
# device program
# ----------------------------------------------------------------------------

def _coloc(insts):
    first = insts[0]
    for x in insts[1:]:
        add_dep_helper(x.ins, first.ins, sync=True, reason="psum coloc order")


def _after(consumer, last_mm):
    """PSUM banks are single-port: a reader of one co-located half must wait
    until the PE is done with the WHOLE bank (fatal collision otherwise)."""
    add_dep_helper(consumer.ins, last_mm.ins, sync=True, reason="bank read-after-all-mm")


def _emit(nc, tc, di, d_out, W, OFF, MINACT):
    import os
    KLEVEL = int(os.environ.get("KLEVEL", "3"))
    import contextlib
    ctx = contextlib.ExitStack()
    TOTAL = OFF[-1] + W[-1]
    with ctx:
        singles = ctx.enter_context(tc.tile_pool(name="singles", bufs=1))
        sb2 = ctx.enter_context(tc.tile_pool(name="work2", bufs=2))
        sb3 = ctx.enter_context(tc.tile_pool(name="work3", bufs=3))

        def load(name):
            d = di[name]
            t = singles.tile(list(d.shape), d.dtype, tag=name)
            nc.sync.dma_start(out=t, in_=d.ap())
            return t

        # early loads (scan-phase inputs)
        wihT = load("wihT")
        xintra = load("xintra")
        whhT = [load("whh0T"), load("whh1T")]
        blobD = load("blobD")
        xin = load("xinter")
        ind_all = load("indr")
        rT = load("rT")
        xlast = load("xlast")
        bD = {nm: blobD[:, 2 * i:2 * i + 2] for i, nm in enumerate(_BLOBD_NAMES)}
        b_r, nb_z, b_in, b_hn = bD["b_r"], bD["nb_z"], bD["b_in"], bD["b_hn"]
        aqb, akb = bD["aqb"], bD["akb"]

        # deferred loads (attention weights; DMA emitted here, lands mid-scan
        # behind the big xinter transfer on the sync queue)
        blobB = singles.tile([128, BLOBW], BF16, tag="blobB")
        nc.sync.dma_start(out=blobB, in_=di["blobB"].ap())
        blobC = singles.tile([1, 256 * len(_BLOBC_NAMES)], BF16, tag="blobC")
        nc.sync.dma_start(out=blobC, in_=di["blobC"].ap())
        W_ = {}
        for nm, (o_, w_) in _BLOB_OFF.items():
            rows = 127 if nm in _ROWS127 else (S if nm == "cmask" else 128)
            W_[nm] = blobB[0:rows, o_:o_ + w_]
        for i, nm in enumerate(_BLOBC_NAMES):
            W_[nm] = blobC[:, 256 * i:256 * (i + 1)]

        ones = singles.tile([1, 128], BF16, tag="ones")
        nc.vector.memset(ones, 1.0)

        xn_all = singles.tile([128, 2, TOTAL], BF16, tag="xn_all")
        xn_intra = singles.tile([128, 2, B, S], BF16, tag="xn_intra")
        hT_all = singles.tile([128, 2, B, S], BF16, tag="hT_all")
        zeros16 = singles.tile([128, 2, B], BF16, tag="zeros16")
        nc.vector.memset(zeros16, 0.0)
        h_inter = singles.tile([128, 2, NSEQ], BF16, tag="h_inter")
        nc.vector.memset(h_inter, 0.0)

        # GRU-phase psum pools: rz/zz/nn x2 (6 banks) + ia/ib (2) = 8; the
        # interleaved attention pieces borrow the psg "nn" tag's banks.
        gru_ps = tc.tile_pool(name="psg", bufs=2, space="PSUM")
        psg = gru_ps.__enter__()
        gru_psi = tc.tile_pool(name="psi", bufs=1, space="PSUM")
        psi = gru_psi.__enter__()

        # ---------------- phase 1 pieces: xn = w_ih_n @ x (+b_in via evac) ----
        def xn_inter_step(t):
            for (o, w) in _tiles_of(W[t]):
                xt = xin[:, OFF[t] + o: OFF[t] + o + w]
                px = psg.tile([128, 2, NT], F32, tag="rz")
                m0 = nc.tensor.matmul(px[:, 0, :w], wihT[:, 512:640],
                                      xt, start=True, stop=False)
                m1 = nc.tensor.matmul(px[:, 1, :w], wihT[:, 640:768],
                                      xt, start=False, stop=True)
                _coloc([m0, m1])
                dst = xn_all[:, :, OFF[t] + o: OFF[t] + o + w]
                ev0 = nc.scalar.activation(dst[:, 0, :], px[:, 0, :w], AF.Identity,
                                           bias=b_in[:, 0:1])
                _after(ev0, m1)
                nc.vector.tensor_scalar_add(dst[:, 1, :], px[:, 1, :w], b_in[:, 1:2])

        def xn_intra_all():
            xflat = xintra.rearrange("d b s -> d (b s)")
            for j in range(2):
                o = j * 512
                for ci in range(2):
                    px = psg.tile([128, 512], F32, tag="nn")
                    nc.tensor.matmul(px, wihT[:, 512 + ci * 128: 640 + ci * 128],
                                     xflat[:, o:o + 512], start=True, stop=True)
                    dst = xn_intra.rearrange("p c b s -> p c (b s)")[:, ci, o:o + 512]
                    if ci == 0:
                        nc.scalar.activation(dst, px, AF.Identity, bias=b_in[:, 0:1])
                    else:
                        nc.vector.tensor_scalar_add(dst, px, b_in[:, 1:2])

        # ---------------- phase 2: scans ----------------
        def inter_tile(t, o, w):
            h = h_inter
            freeze = (o + w) > MINACT[t]
            rz = psg.tile([128, 2, NT], F32, tag="rz")
            zz = psg.tile([128, 2, NT], F32, tag="zz")
            nn = psg.tile([128, 2, NT], F32, tag="nn")
            xt = xin[:, OFF[t] + o: OFF[t] + o + w]

            def gate_bank(ps, g0, fz):
                insts = []
                last = None
                for ci in range(2):
                    g = g0 + ci
                    sl = slice(g * 128, (g + 1) * 128)
                    mm = nc.tensor.matmul(ps[:, ci, :w], wihT[:, sl], xt,
                                          start=(ci == 0), stop=False)
                    insts.append(mm)
                    nc.tensor.matmul(ps[:, ci, :w], whhT[0][:, sl], h[:, 0, o:o + w],
                                     start=False, stop=False)
                    last = nc.tensor.matmul(ps[:, ci, :w], whhT[1][:, sl],
                                            h[:, 1, o:o + w],
                                            start=False, stop=(not fz) and ci == 1)
                    if fz:
                        last = nc.tensor.matmul(
                            ps[:, ci, :w], ones,
                            ind_all[:, OFF[t] + o: OFF[t] + o + w],
                            start=False, stop=(ci == 1))
                _coloc(insts)
                return last

            rz_last = gate_bank(rz, 0, False)
            zz_last = gate_bank(zz, 2, freeze)
            i0 = nc.tensor.matmul(nn[:, 0, :w], whhT[0][:, 512:640], h[:, 0, o:o + w],
                                  start=True, stop=False)
            nc.tensor.matmul(nn[:, 0, :w], whhT[1][:, 512:640], h[:, 1, o:o + w],
                             start=False, stop=False)
            i1 = nc.tensor.matmul(nn[:, 1, :w], whhT[0][:, 640:768], h[:, 0, o:o + w],
                                  start=False, stop=False)
            nn_last = nc.tensor.matmul(nn[:, 1, :w], whhT[1][:, 640:768],
                                       h[:, 1, o:o + w], start=False, stop=True)
            _coloc([i0, i1])

            r_sb = sb3.tile([128, 2, NT], BF16, tag="r_sb")
            zc_sb = sb3.tile([128, 2, NT], BF16, tag="zc_sb")
            t1_sb = sb3.tile([128, 2, NT], BF16, tag="t1_sb")
            u_sb = sb3.tile([128, 2, NT], BF16, tag="u_sb")
            n_sb = sb3.tile([128, 2, NT], BF16, tag="n_sb")
            d_sb = sb3.tile([128, 2, NT], BF16, tag="d_sb")
            f_sb = sb3.tile([128, 2, NT], BF16, tag="f_sb")
            for ci in range(2):
                _after(nc.scalar.activation(r_sb[:, ci, :w], rz[:, ci, :w], AF.Sigmoid,
                                            bias=b_r[:, ci:ci + 1]), rz_last)
                _after(nc.scalar.activation(zc_sb[:, ci, :w], zz[:, ci, :w], AF.Sigmoid,
                                            bias=nb_z[:, ci:ci + 1], scale=-1.0),
                       zz_last)
                _after(nc.vector.scalar_tensor_tensor(
                    t1_sb[:, ci, :w], nn[:, ci, :w], b_hn[:, ci:ci + 1], r_sb[:, ci, :w],
                    op0=ALU.add, op1=ALU.mult), nn_last)
            nc.vector.tensor_add(u_sb[:, :, :w], t1_sb[:, :, :w],
                                 xn_all[:, :, OFF[t] + o: OFF[t] + o + w])
            nc.scalar.activation(n_sb[:, :, :w], u_sb[:, :, :w], AF.Tanh)
            hsl = h[:, :, o:o + w]
            nc.gpsimd.tensor_sub(d_sb[:, :, :w], hsl, n_sb[:, :, :w])
            nc.gpsimd.tensor_mul(f_sb[:, :, :w], zc_sb[:, :, :w], d_sb[:, :, :w])
            nc.vector.tensor_sub(hsl, hsl, f_sb[:, :, :w])

        def intra_step(s):
            hprev = zeros16 if s == 0 else hT_all[:, :, :, s - 1]
            ia = psi.tile([128, 4, B], F32, tag="ia")
            ib = psi.tile([128, 2, B], F32, tag="ib")
            xt = xintra[:, :, s]
            insts = []
            ia_last = None
            for g in range(4):
                sl = slice(g * 128, (g + 1) * 128)
                mm = nc.tensor.matmul(ia[:, g, :], wihT[:, sl], xt,
                                      start=(g == 0), stop=False)
                insts.append(mm)
                nc.tensor.matmul(ia[:, g, :], whhT[0][:, sl], hprev[:, 0, :],
                                 start=False, stop=False)
                ia_last = nc.tensor.matmul(ia[:, g, :], whhT[1][:, sl], hprev[:, 1, :],
                                           start=False, stop=(g == 3))
            _coloc(insts)
            insts = []
            ib_last = None
            for ci in range(2):
                sl = slice(512 + ci * 128, 512 + (ci + 1) * 128)
                mm = nc.tensor.matmul(ib[:, ci, :], whhT[0][:, sl], hprev[:, 0, :],
                                      start=(ci == 0), stop=False)
                insts.append(mm)
                ib_last = nc.tensor.matmul(ib[:, ci, :], whhT[1][:, sl], hprev[:, 1, :],
                                           start=False, stop=(ci == 1))
            _coloc(insts)

            r_sb = sb2.tile([128, 2, B], BF16, tag="ir_sb")
            zc_sb = sb2.tile([128, 2, B], BF16, tag="izc_sb")
            t1_sb = sb2.tile([128, 2, B], BF16, tag="it1_sb")
            u_sb = sb2.tile([128, 2, B], BF16, tag="iu_sb")
            n_sb = sb2.tile([128, 2, B], BF16, tag="in_sb")
            d_sb = sb2.tile([128, 2, B], BF16, tag="id_sb")
            f_sb = sb2.tile([128, 2, B], BF16, tag="if_sb")
            for ci in range(2):
                _after(nc.scalar.activation(r_sb[:, ci, :], ia[:, ci, :], AF.Sigmoid,
                                            bias=b_r[:, ci:ci + 1]), ia_last)
                _after(nc.scalar.activation(zc_sb[:, ci, :], ia[:, 2 + ci, :],
                                            AF.Sigmoid, bias=nb_z[:, ci:ci + 1],
                                            scale=-1.0), ia_last)
                _after(nc.vector.scalar_tensor_tensor(
                    t1_sb[:, ci, :], ib[:, ci, :], b_hn[:, ci:ci + 1], r_sb[:, ci, :],
                    op0=ALU.add, op1=ALU.mult), ib_last)
            nc.vector.tensor_add(u_sb, t1_sb, xn_intra[:, :, :, s])
            nc.scalar.activation(n_sb, u_sb, AF.Tanh)
            nc.gpsimd.tensor_sub(d_sb, hprev, n_sb)
            nc.gpsimd.tensor_mul(f_sb, zc_sb, d_sb)
            nc.vector.tensor_sub(hT_all[:, :, :, s], hprev, f_sb)

        # ---------------- interleaved attention pieces ----------------
        k_sb = singles.tile([128, NST, 256], BF16, tag="k_sb")
        v_sb = singles.tile([128, NST, 256], BF16, tag="v_sb")
        qa_sb = singles.tile([128, 2, 128], BF16, tag="qa_sb")
        ka_sb = singles.tile([128, 2, 128], BF16, tag="ka_sb")
        paT_all = singles.tile([S, BPC, 2, S], BF16, tag="paT_all")

        hflat = hT_all.rearrange("p c b s -> p c (b s)")   # [128, 2, 1024]
        hown = [hflat[:, ci, 0:NTOK] for ci in range(2)]    # [128, 128] each
        xflat_i = xintra.rearrange("d b s -> d (b s)")
        xp_own = xflat_i[0:127, 0:NTOK]                     # [127, 128]

        def proj_psx(lhs_chunks, rhs_tiles, bias_tile, out_sb):
            p = psg.tile([128, 2, NT], F32, tag="nn", name="p")[:, 0, :]
            first = True
            for (lt, rt) in zip(lhs_chunks, rhs_tiles):
                nc.tensor.matmul(p, lt, rt, start=first, stop=False)
                first = False
            nc.tensor.matmul(p, ones, bias_tile, start=False, stop=True)
            nc.scalar.copy(out_sb, p)

        def kv_tile(s_):
            cols = slice(s_ * 128, (s_ + 1) * 128)
            proj_psx([h_inter[:, 0, cols], h_inter[:, 1, cols], rT[0:127, cols]],
                     [W_["ikw0"], W_["ikw1"], W_["ikwx"]], W_["ikb"], k_sb[:, s_, :])
            proj_psx([h_inter[:, 0, cols], h_inter[:, 1, cols], rT[:, cols]],
                     [W_["ivw0"], W_["ivw1"], W_["ivwx"]], W_["ivb"], v_sb[:, s_, :])

        def qaka():
            for wn, ob, bias in (("aqw", qa_sb, aqb), ("akw", ka_sb, akb)):
                ps = psg.tile([128, 2, NT], F32, tag="nn", name="ps")[:, :, 0:128]
                insts = []
                for ci in range(2):
                    mm = nc.tensor.matmul(ps[:, ci, :],
                                          W_[wn][:, ci * 128:(ci + 1) * 128],
                                          xp_own, start=(ci == 0), stop=(ci == 1))
                    insts.append(mm)
                _coloc(insts)
                for ci in range(2):
                    _after(nc.scalar.activation(ob[:, ci, :], ps[:, ci, :],
                                                AF.Identity, bias=bias[:, ci:ci + 1]),
                           insts[-1])

        def intra_chain(bl, hh):
            sca = psg.tile([128, 2, NT], F32, tag="nn", name="sca")[0:S, 0, 0:S]
            mm = nc.tensor.matmul(sca, qa_sb[:, hh, bl * S:(bl + 1) * S],
                                  ka_sb[:, hh, bl * S:(bl + 1) * S],
                                  start=True, stop=True)
            ms = sb3.tile([S, S], BF16, tag="ms")
            nc.vector.tensor_add(ms, sca, W_["cmask"])
            ex = sb3.tile([S, S], BF16, tag="ex")
            nc.scalar.activation(ex, ms, AF.Exp)
            rs = sb3.tile([S, 1], F32, tag="rs")
            nc.vector.tensor_reduce(rs, ex, axis=AX.X, op=ALU.add)
            ri = sb3.tile([S, 1], F32, tag="ri")
            nc.vector.reciprocal(ri, rs)
            pa = sb3.tile([S, S], BF16, tag="pa")
            nc.vector.tensor_scalar_mul(pa, ex, ri)
            ptp = psg.tile([S, S], BF16, tag="nn", name="ptp")
            nc.tensor.transpose(ptp, pa, W_["id128"][0:S, 0:S])
            nc.vector.tensor_copy(paT_all[:, bl, hh, :], ptp)

        # k/v tiles become final when the active width drops below their cols
        kv_after_step = {}
        for s_ in range(NST):
            ready = max((t for t in range(L) if W[t] > s_ * 128), default=-1)
            kv_after_step.setdefault(ready, []).append(s_)

        # ---------------- emission schedule ----------------
        xn_intra_all()
        XN_LEAD = 6
        for t in range(XN_LEAD):
            xn_inter_step(t)

        if KLEVEL == 1:
            ob = sb2.tile([128, 256], F32, tag="out_sb", name="ob")
            nc.vector.tensor_copy(ob, xn_all[:, 0, 0:256])
            nc.sync.dma_start(out=d_out.ap(), in_=ob)
            gru_psx.__exit__(None, None, None)
            gru_psi.__exit__(None, None, None)
            gru_ps.__exit__(None, None, None)
            return

        inter_iters = [(t, o, w) for t in range(L) for (o, w) in _tiles_of(W[t])]
        NT_TOT = len(inter_iters)
        emitted = 0
        xn_done = XN_LEAD
        kv_q = []
        kv_done = set()
        for i in range(S):
            intra_step(i)
            if i == 40:
                qaka()
            if i == 54:
                for bl in range(BPC):
                    for hh in range(2):
                        intra_chain(bl, hh)
            # drain ready k/v projections into the thin late-scan window
            if i >= 44 and kv_q:
                kv_tile(kv_q.pop(0))
            while xn_done < L and xn_done < XN_LEAD + (i * (L - XN_LEAD)) // 45:
                xn_inter_step(xn_done)
                xn_done += 1
            target = min(NT_TOT, ((i + 1) * NT_TOT) // S)
            while emitted < target:
                t, o, w = inter_iters[emitted]
                assert t < max(xn_done, XN_LEAD)
                inter_tile(t, o, w)
                emitted += 1
                last_of_step = (emitted == NT_TOT or inter_iters[emitted][0] != t)
                if last_of_step:
                    for s_ in kv_after_step.get(t, []):
                        kv_q.append(s_)
                        kv_done.add(s_)
        his_last = h_inter
        for s_ in kv_q + [s_ for s_ in range(NST) if s_ not in kv_done]:
            kv_tile(s_)
        gru_psi.__exit__(None, None, None)
        gru_ps.__exit__(None, None, None)

        if KLEVEL == 2:
            ob = sb2.tile([128, 256], F32, tag="out_sb", name="ob")
            nc.vector.tensor_copy(ob[:, 0:128], his_last[:, 0, 0:128])
            nc.vector.tensor_copy(ob[:, 128:256], hflat[:, 0, 0:128])
            nc.sync.dma_start(out=d_out.ap(), in_=ob)
            return

        # ---------------- phase 3: attention + fused final ----------------
        psa = ctx.enter_context(tc.tile_pool(name="psa", bufs=2, space="PSUM"))
        psb = ctx.enter_context(tc.tile_pool(name="psb", bufs=2, space="PSUM"))
        psf = ctx.enter_context(tc.tile_pool(name="psf", bufs=1, space="PSUM"))

        def proj(lhs_chunks, rhs_tiles, bias_tile, m_parts=128):
            p = psa.tile([m_parts, 256], F32, tag="proj")
            first = True
            for (lt, rt) in zip(lhs_chunks, rhs_tiles):
                nc.tensor.matmul(p, lt, rt, start=first, stop=False)
                first = False
            nc.tensor.matmul(p, ones[:, 0:m_parts], bias_tile, start=False, stop=True)
            return p

        q_ps = proj([hown[0], hown[1], xp_own],
                    [W_["iqw0"], W_["iqw1"], W_["iqwx"]], W_["iqb"])
        q_sb = sb2.tile([128, 256], BF16, tag="q_sb")
        nc.scalar.copy(q_sb, q_ps)

        # sorted-frame attention: q permute + scores + exp per 128-seq tile
        e_sb = singles.tile([128, NST, 2], BF16, tag="e_sb")
        e32 = singles.tile([128, NST, 2], F32, tag="e32")
        for s_ in range(NST):
            cols = slice(s_ * 128, (s_ + 1) * 128)
            qpp = psa.tile([128, 256], F32, tag="proj")
            nc.tensor.matmul(qpp, W_["Pq"][:, cols], q_sb, start=True, stop=True)
            qp_sb = sb3.tile([128, 256], BF16, tag="qp_sb")
            nc.scalar.copy(qp_sb, qpp)
            scratch = sb3.tile([128, 2, 128], BF16, tag="ttr_scratch")
            nc.vector.tensor_mul(scratch,
                                 qp_sb.rearrange("p (c n) -> p c n", c=2),
                                 k_sb[:, s_, :].rearrange("p (c n) -> p c n", c=2))
            sc = sb3.tile([128, 2, 1], F32, tag="sc")
            nc.vector.tensor_reduce(sc, scratch, axis=AX.X, op=ALU.add)
            nc.scalar.activation(e32[:, s_, :].rearrange("p (c one) -> p c one", c=2),
                                 sc, AF.Exp)
            nc.vector.tensor_copy(e_sb[:, s_, :], e32[:, s_, :])

        # esum per token + weighted values, co-located in one PSUM bank:
        # esum at cols 256:258, o at cols 0:256; single start=True clears bank.
        acc = psf.tile([128, 512], F32, tag="acc")
        esum_ps = acc[:, 256:258]
        o_ps = acc[:, 0:256]
        acc_insts = []
        for s_ in range(NST):
            cols = slice(s_ * 128, (s_ + 1) * 128)
            acc_insts.append(nc.tensor.matmul(
                esum_ps, W_["Pi"][:, cols], e_sb[:, s_, :],
                start=(s_ == 0), stop=False))
        ow_last = None
        for s_ in range(NST):
            cols = slice(s_ * 128, (s_ + 1) * 128)
            vw = sb3.tile([128, 256], BF16, tag="vw")
            for hh in range(2):
                hs = slice(hh * 128, (hh + 1) * 128)
                nc.vector.tensor_scalar_mul(vw[:, hs], v_sb[:, s_, hs],
                                            e32[:, s_, hh:hh + 1])
            ow_last = nc.tensor.matmul(o_ps, W_["Pi"][:, cols], vw,
                                       start=False, stop=(s_ == NST - 1))
            acc_insts.append(ow_last)
        _coloc(acc_insts)
        einv = sb2.tile([128, 2], F32, tag="einv")
        _after(nc.vector.reciprocal(einv, esum_ps), ow_last)
        o_i = sb2.tile([128, 256], BF16, tag="o_i")
        for hh in range(2):
            hs = slice(hh * 128, (hh + 1) * 128)
            _after(nc.vector.tensor_scalar_mul(o_i[:, hs], o_ps[:, hs],
                                               einv[:, hh:hh + 1]), ow_last)

        if KLEVEL == 27:
            ob = sb2.tile([128, 256], F32, tag="out_sb", name="ob")
            nc.vector.tensor_copy(ob, o_i)
            nc.sync.dma_start(out=d_out.ap(), in_=ob)
            return

        oiT = sb2.tile([128, 2, 128], BF16, tag="oiT")
        for ci in range(2):
            tp = psb.tile([128, 128], BF16, tag="tp", name="tp")
            nc.tensor.transpose(tp, o_i[:, ci * 128:(ci + 1) * 128], W_["id128"])
            nc.vector.tensor_copy(oiT[:, ci, :], tp)

        va_sb = []
        for bl in range(BPC):
            vp = proj([hT_all[:, 0, bl, :], hT_all[:, 1, bl, :], xlast[:, bl, :]],
                      [W_["avw0"], W_["avw1"], W_["avwx"]], W_["avb"], m_parts=S)
            vb = sb2.tile([S, 256], BF16, tag="va_sb")
            nc.scalar.copy(vb, vp)
            va_sb.append(vb)

        oaT = sb2.tile([128, 2, 128], BF16, tag="oaT")
        for bl in range(BPC):
            for hh in range(2):
                op = psb.tile([128, S], F32, tag="tp")
                nc.tensor.matmul(op, va_sb[bl][:, hh * 128:(hh + 1) * 128],
                                 paT_all[:, bl, hh, :], start=True, stop=True)
                nc.vector.tensor_copy(oaT[:, hh, bl * S:(bl + 1) * S], op)

        if KLEVEL == 29:
            ob = sb2.tile([128, 256], F32, tag="out_sb", name="ob")
            nc.vector.tensor_copy(ob[:, 0:128], oaT[:, 0, :])
            nc.vector.tensor_copy(ob[0:64, 128:256], va_sb[0][:, 0:128])
            nc.sync.dma_start(out=d_out.ap(), in_=ob[:, :])
            return

        # fused final projection
        fo = psf.tile([128, 512], F32, tag="acc", name="fo")[:, 0:256]
        nc.tensor.matmul(fo, oiT[:, 0, :], W_["AiT0"], start=True, stop=False)
        nc.tensor.matmul(fo, oiT[:, 1, :], W_["AiT1"], start=False, stop=False)
        nc.tensor.matmul(fo, oaT[:, 0, :], W_["AaT0"], start=False, stop=False)
        nc.tensor.matmul(fo, oaT[:, 1, :], W_["AaT1"], start=False, stop=False)
        nc.tensor.matmul(fo, hown[0], W_["LhT0"], start=False, stop=False)
        nc.tensor.matmul(fo, hown[1], W_["LhT1"], start=False, stop=False)
        nc.tensor.matmul(fo, xp_own, W_["LxT"], start=False, stop=False)
        nc.tensor.matmul(fo, ones, W_["btot"], start=False, stop=True)
        out_sb = sb2.tile([128, 256], F32, tag="out_sb")
        nc.vector.tensor_copy(out_sb, fo)
        nc.sync.dma_start(out=d_out.ap(), in_=out_sb)


def _build(W, MINACT):
    OFF = [0]
    for t in range(L - 1):
        OFF.append(OFF[t] + W[t])
    TOTAL = OFF[-1] + W[-1]

    nc = bacc.Bacc("TRN2", target_bir_lowering=False, debug=False)
    di = {}

    def inp(name, shape, dt=BF16):
        di[name] = nc.dram_tensor(name, list(shape), dt, kind="ExternalInput")

    inp("xinter", [128, TOTAL])
    inp("xintra", [128, B, S])
    inp("xlast", [1, B, S])
    inp("rT", [128, NSEQ])
    inp("indr", [1, TOTAL])
    inp("wihT", [128, 768])
    inp("whh0T", [128, 768])
    inp("whh1T", [128, 768])
    inp("blobB", [128, BLOBW])
    inp("blobC", [1, 256 * len(_BLOBC_NAMES)])
    inp("blobD", [128, 2 * len(_BLOBD_NAMES)], F32)

    d_out = nc.dram_tensor("out", [NTOK, 256], F32, kind="ExternalOutput")

    with tile.TileContext(nc) as tc:
        _emit(nc, tc, di, d_out, W, OFF, MINACT)
    nc.compile()
    return nc


# ----------------------------------------------------------------------------
# host-side prep
# ----------------------------------------------------------------------------

def _plan(inter_len):
    """Per-core length-sort plan + shared compile-time widths."""
    lens5 = np.asarray(inter_len, np.int64).reshape(B, S, R)
    orders, lens_sorted = [], []
    act = np.zeros((NCORES, L), np.int64)
    for c in range(NCORES):
        lens = lens5[[2 * c, 2 * c + 1]].reshape(NSEQ)
        order = np.argsort(-lens, kind="stable")
        ls = lens[order]
        orders.append(order)
        lens_sorted.append(ls)
        for t in range(L):
            act[c, t] = int((ls > t).sum())
    W = [min(NSEQ, int(-32 * (-(act[:, t].max()) // 32))) for t in range(L)]
    W = [max(32, w) for w in W]
    for t in range(1, L):
        W[t] = min(W[t], W[t - 1])
    MINACT = [int(act[:, t].min()) for t in range(L)]
    OFF = [0]
    for t in range(L - 1):
        OFF.append(OFF[t] + W[t])
    return orders, lens_sorted, W, MINACT, OFF


def prep_in_maps(inputs):
    inp = {k: np.asarray(v) for k, v in inputs.items()}
    w_ih = f32c(inp["w_ih"])
    w_hh = f32c(inp["w_hh"])
    b_ih = f32c(inp["b_ih"])
    b_hh = f32c(inp["b_hh"])
    b_rz = b_ih[:2 * H] + b_hh[:2 * H]
    sq = np.sqrt(128.0)

    e = np.exp(f32c(inp["wr"])[0, 0] - f32c(inp["wr"])[0, 0].max())
    w01 = e / e.sum()
    ln_w = f32c(inp["ln_w"])
    L_v, L_h, L_x = ln_w[:, :H], ln_w[:, H:2 * H], ln_w[:, 2 * H:]
    Ai = w01[0] * (L_v @ f32c(inp["io_w"]))
    Aa = w01[1] * (L_v @ f32c(inp["ao_w"]))
    btot = f32c(inp["ln_b"]) + L_v @ (w01[0] * f32c(inp["io_b"]) + w01[1] * f32c(inp["ao_b"]))

    iq_w = f32c(inp["iq_w"]) / sq
    iq_b = f32c(inp["iq_b"]) / sq
    aq_w = f32c(inp["aq_w"]) / sq
    aq_b = f32c(inp["aq_b"]) / sq

    def chunks2(m):  # [128,2] fp32 per-partition chunk tiles
        return f32c(np.stack([m[:128], m[128:256]], axis=1))

    orders, lens_sorted, W, MINACT, OFF = _plan(inp["inter_len"])
    TOTAL = OFF[-1] + W[-1]

    x_bs = f32c(inp["intra_x"])                     # [B,S,D]
    his5 = f32c(inp["inter_his"]).reshape(B, S, R, L, D)
    r5 = f32c(inp["inter_r"]).reshape(B, S, R, D)

    # shared weight blobs
    bw = {
        "iqw0": iq_w.T[0:128], "iqw1": iq_w.T[128:256], "iqwx": iq_w.T[256:383],
        "ikw0": inp["ik_w"].T[0:128], "ikw1": inp["ik_w"].T[128:256],
        "ikwx": inp["ik_w"].T[256:383],
        "ivw0": inp["iv_w"].T[0:128], "ivw1": inp["iv_w"].T[128:256],
        "ivwx": inp["iv_w"].T[256:384],
        "aqw": aq_w.T, "akw": f32c(inp["ak_w"]).T,
        "avw0": inp["av_w"].T[0:128], "avw1": inp["av_w"].T[128:256],
        "AiT0": Ai.T[0:128], "AiT1": Ai.T[128:256],
        "AaT0": Aa.T[0:128], "AaT1": Aa.T[128:256],
        "LhT0": L_h.T[0:128], "LhT1": L_h.T[128:256], "LxT": L_x.T,
        "id128": np.eye(128, dtype=np.float32),
        "cmask": np.where(np.tril(np.ones((S, S), bool)), 0.0, -BIG),
    }
    blobC = np.zeros((1, 256 * len(_BLOBC_NAMES)), np.float32)
    bc = {
        "iqb": iq_b, "ikb": f32c(inp["ik_b"]), "ivb": f32c(inp["iv_b"]),
        "avwx": f32c(inp["av_w"]).T[256], "avb": f32c(inp["av_b"]), "btot": btot,
    }
    for i, nm in enumerate(_BLOBC_NAMES):
        blobC[0, 256 * i:256 * i + len(bc[nm])] = bc[nm]
    blobD = np.zeros((128, 2 * len(_BLOBD_NAMES)), np.float32)
    bd = {
        "b_r": chunks2(b_rz[:H]), "nb_z": chunks2(-b_rz[H:]),
        "b_in": chunks2(b_ih[2 * H:]), "b_hn": chunks2(b_hh[2 * H:]),
        "aqb": chunks2(aq_b), "akb": chunks2(f32c(inp["ak_b"])),
    }
    for i, nm in enumerate(_BLOBD_NAMES):
        blobD[:, 2 * i:2 * i + 2] = bd[nm]

    shared = dict(
        wihT=bfc(w_ih.T),
        whh0T=bfc(w_hh.T[0:128]),
        whh1T=bfc(w_hh.T[128:256]),
        blobC=bfc(blobC),
        blobD=f32c(blobD),
    )

    in_maps = []
    for c in range(NCORES):
        bsel = [2 * c, 2 * c + 1]
        order = orders[c]
        ls = lens_sorted[c]
        # inter: sorted seq order; orig col order is ((bl,s),r)
        his_cols = his5[bsel].reshape(NSEQ, L, D)[order]    # [NSEQ, L, D]
        xint = np.zeros((D, TOTAL), np.float32)
        ind = np.zeros((1, TOTAL), np.float32)
        for t in range(L):
            o, w = OFF[t], W[t]
            xint[:, o:o + w] = his_cols[:w, t, :].T
            ind[0, o:o + w] = BIG * (t >= ls[:w])
        rTc = r5[bsel].reshape(NSEQ, D)[order].T            # [D, NSEQ]
        # permutation matrices: tok_of[p] for sorted position p
        tok_of = order // R
        Pq = np.zeros((128, NSEQ), np.float32)
        Pi = np.zeros((128, NSEQ), np.float32)
        for s_ in range(NST):
            for pl in range(128):
                tok = tok_of[s_ * 128 + pl]
                Pq[tok, s_ * 128 + pl] = 1.0
                Pi[pl, s_ * 128 + tok] = 1.0
        blobB = np.zeros((128, BLOBW), np.float32)
        for nm, (o_, w_) in _BLOB_OFF.items():
            src = {"Pq": Pq, "Pi": Pi}.get(nm)
            if src is None:
                src = bw[nm]
            blobB[0:src.shape[0], o_:o_ + src.shape[1]] = src
        # intra: batches rotated so own batches are 0..1; (d, b, s) layout
        rolled = np.roll(x_bs, -2 * c, axis=0)
        xia = rolled.transpose(2, 0, 1)             # [D, B, S]
        m = dict(shared)
        m.update(
            xinter=bfc(xint),
            xintra=bfc(xia),
            xlast=bfc(xia[127:128]),
            rT=bfc(rTc),
            indr=bfc(ind),
            blobB=bfc(blobB),
        )
        in_maps.append(m)
    return in_maps, W, MINACT


def assemble(core_outs):
    o = np.stack([np.asarray(co, np.float32) for co in core_outs])  # [8,128,256]
    return np.ascontiguousarray(o.reshape(B * S, 256))


_CACHE = {}


def kernel(**inputs) -> np.ndarray:
    in_maps, W, MINACT = prep_in_maps(inputs)
    key = (tuple(W), tuple(MINACT))
    if _CACHE.get("key") != key:
        _CACHE["nc"] = _build(W, MINACT)
        _CACHE["key"] = key
    nc = _CACHE["nc"]
    res = run_bass_kernel_spmd(nc, in_maps, core_ids=list(range(NCORES)))
    return assemble([r["out"] for r in res.results])


# revision 30
# speedup vs baseline: 1.2101x; 1.0502x over previous
"""Trainium2 Bass kernel for nn_CoKT (dual GRU + cross/causal attention + fused linear).

Self-contained: builds an 8-core SPMD Tile kernel, shards tokens (B*S) across
cores (2 batches/core), replicates weights, runs via run_bass_kernel_spmd,
reassembles the full [1024, 256] fp32 output.

Per-core design (128 own tokens, core-local order (bl, s)):
- inter GRU: 768 seqs x 24 steps, seqs GLOBALLY SORTED by length (descending)
  on host; step t only processes the active prefix W(t) = max-over-cores
  active count rounded to 32 (z-freeze +BIG trick protects the padded tail of
  each boundary tile). Cuts seq-step work ~1.8x vs dense. h updated in place.
- attention runs in the sorted frame: q is permuted per 128-seq tile with
  host-built permutation matmuls; softmax normalization and the output
  accumulation go back to token frame via indicator matmuls.
- k/v projections of finished seq-tiles and the intra-attention score/softmax
  chains are interleaved INTO the scan (they only depend on frozen h columns
  resp. xintra), shrinking the serial tail.
- intra GRU: batch 16 x 64 steps, replicated on every core (weight-load bound
  either way); host rotates batches so own 2 batches are columns 0..1.
- xinter/ind fully SBUF-resident (one DMA each); attention weights packed
  into one blob DMA'd mid-scan.
"""
import sys
if "/opt/trn_rl_repo" not in sys.path:
    sys.path.insert(0, "/opt/trn_rl_repo")

import numpy as np
import ml_dtypes

import concourse.bacc as bacc
import concourse.mybir as mybir
import concourse.tile as tile
from concourse.tile import add_dep_helper
from concourse.bass_utils import run_bass_kernel_spmd

F32 = mybir.dt.float32
BF16 = mybir.dt.bfloat16
AF = mybir.ActivationFunctionType
ALU = mybir.AluOpType
AX = mybir.AxisListType

B, S, R, L, D, H = 16, 64, 6, 24, 128, 256
NCORES = 8
BPC = B // NCORES            # 2 batches per core
NTOK = S * BPC               # 128 own tokens
NSEQ = NTOK * R              # 768 inter sequences per core
NT = 256                     # inter token-tile width
BIG = 30000.0
NST = NSEQ // 128            # 6 seq-tiles of 128 in the attention phase

bfc = lambda x: np.ascontiguousarray(np.asarray(x, np.float32).astype(ml_dtypes.bfloat16))
f32c = lambda x: np.ascontiguousarray(np.asarray(x, np.float32))

# attention-weight blob layout: name -> (col offset, width); all 128 rows
# (127-row tensors sit in rows 0:127, cmask in rows 0:64).
_BLOB_NAMES = [
    ("iqw0", 256), ("iqw1", 256), ("ikw0", 256), ("ikw1", 256),
    ("ivw0", 256), ("ivw1", 256), ("ivwx", 256), ("avw0", 256), ("avw1", 256),
    ("AiT0", 256), ("AiT1", 256), ("AaT0", 256), ("AaT1", 256),
    ("LhT0", 256), ("LhT1", 256),
    ("iqwx", 256), ("ikwx", 256), ("aqw", 256), ("akw", 256), ("LxT", 256),
    ("id128", 128), ("Pq", NSEQ), ("Pi", NSEQ), ("cmask", S),
]
_BLOB_OFF = {}
_off = 0
for _nm, _w in _BLOB_NAMES:
    _BLOB_OFF[_nm] = (_off, _w)
    _off += _w
BLOBW = _off
_ROWS127 = {"iqwx", "ikwx", "aqw", "akw", "LxT"}

_BLOBC_NAMES = ["iqb", "ikb", "ivb", "avwx", "avb", "btot"]
_BLOBD_NAMES = ["b_r", "nb_z", "b_in", "b_hn", "aqb", "akb"]


def _tiles_of(w):
    out = []
    o = 0
    while o < w:
        out.append((o, min(NT, w - o)))
        o += NT
    return out


# ----------------------------------------------------------------------------
# device program
# ----------------------------------------------------------------------------

def _coloc(insts):
    first = insts[0]
    for x in insts[1:]:
        add_dep_helper(x.ins, first.ins, sync=True, reason="psum coloc order")


def _after(consumer, last_mm):
    """PSUM banks are single-port: a reader of one co-located half must wait
    until the PE is done with the WHOLE bank (fatal collision otherwise)."""
    add_dep_helper(consumer.ins, last_mm.ins, sync=True, reason="bank read-after-all-mm")


def _emit(nc, tc, di, d_out, W, OFF, MINACT):
    import os
    KLEVEL = int(os.environ.get("KLEVEL", "3"))
    import contextlib
    ctx = contextlib.ExitStack()
    TOTAL = OFF[-1] + W[-1]
    with ctx:
        singles = ctx.enter_context(tc.tile_pool(name="singles", bufs=1))
        sb2 = ctx.enter_context(tc.tile_pool(name="work2", bufs=2))
        sb3 = ctx.enter_context(tc.tile_pool(name="work3", bufs=3))

        def load(name):
            d = di[name]
            t = singles.tile(list(d.shape), d.dtype, tag=name)
            nc.sync.dma_start(out=t, in_=d.ap())
            return t

        # early loads (scan-phase inputs)
        wihT = load("wihT")
        xintra = load("xintra")
        whhT = [load("whh0T"), load("whh1T")]
        blobD = load("blobD")
        id128e = load("id128e")
        xin = load("xinter")
        ind_all = load("indr")
        rT = load("rT")
        xlast = load("xlast")
        bD = {nm: blobD[:, 2 * i:2 * i + 2] for i, nm in enumerate(_BLOBD_NAMES)}
        b_r, nb_z, b_in, b_hn = bD["b_r"], bD["nb_z"], bD["b_in"], bD["b_hn"]
        aqb, akb = bD["aqb"], bD["akb"]

        # deferred loads (attention weights; DMA emitted here, lands mid-scan
        # behind the big xinter transfer on the sync queue)
        blobB = singles.tile([128, BLOBW], BF16, tag="blobB")
        nc.sync.dma_start(out=blobB, in_=di["blobB"].ap())
        blobC = singles.tile([1, 256 * len(_BLOBC_NAMES)], BF16, tag="blobC")
        nc.sync.dma_start(out=blobC, in_=di["blobC"].ap())
        W_ = {}
        for nm, (o_, w_) in _BLOB_OFF.items():
            rows = 127 if nm in _ROWS127 else (S if nm == "cmask" else 128)
            W_[nm] = blobB[0:rows, o_:o_ + w_]
        for i, nm in enumerate(_BLOBC_NAMES):
            W_[nm] = blobC[:, 256 * i:256 * (i + 1)]
        W_["id128"] = id128e

        ones = singles.tile([1, 128], BF16, tag="ones")
        nc.vector.memset(ones, 1.0)

        xn_all = singles.tile([128, 2, TOTAL], BF16, tag="xn_all")
        xn_intra = singles.tile([128, 2, B, S], BF16, tag="xn_intra")
        xw4 = singles.tile([128, 4, B, S], BF16, tag="xw4")
        hT_all = singles.tile([128, 2, B, S], BF16, tag="hT_all")
        zeros16 = singles.tile([128, 2, B], BF16, tag="zeros16")
        nc.vector.memset(zeros16, 0.0)
        h_inter = singles.tile([128, 2, NSEQ], BF16, tag="h_inter")
        nc.vector.memset(h_inter, 0.0)

        # GRU-phase psum pools: rz/zz/nn x2 (6 banks) + ia/ib (2) = 8; the
        # interleaved attention pieces borrow the psg "nn" tag's banks.
        gru_ps = tc.tile_pool(name="psg", bufs=2, space="PSUM")
        psg = gru_ps.__enter__()
        gru_psi = tc.tile_pool(name="psi", bufs=1, space="PSUM")
        psi = gru_psi.__enter__()

        # ---------------- phase 1 pieces: xn = w_ih_n @ x (+b_in via evac) ----
        def xn_inter_step(t):
            for (o, w) in _tiles_of(W[t]):
                xt = xin[:, OFF[t] + o: OFF[t] + o + w]
                px = psg.tile([128, 2, NT], F32, tag="rz")
                m0 = nc.tensor.matmul(px[:, 0, :w], wihT[:, 512:640],
                                      xt, start=True, stop=False)
                m1 = nc.tensor.matmul(px[:, 1, :w], wihT[:, 640:768],
                                      xt, start=False, stop=True)
                _coloc([m0, m1])
                dst = xn_all[:, :, OFF[t] + o: OFF[t] + o + w]
                ev0 = nc.vector.tensor_scalar_add(dst[:, 0, :], px[:, 0, :w],
                                                  b_in[:, 0:1])
                _after(ev0, m1)
                nc.vector.tensor_scalar_add(dst[:, 1, :], px[:, 1, :w], b_in[:, 1:2])

        def xn_intra_all():
            xflat = xintra.rearrange("d b s -> d (b s)")
            for j in range(2):
                o = j * 512
                for ci in range(2):
                    px = psg.tile([128, 512], F32, tag="nn")
                    nc.tensor.matmul(px, wihT[:, 512 + ci * 128: 640 + ci * 128],
                                     xflat[:, o:o + 512], start=True, stop=True)
                    dst = xn_intra.rearrange("p c b s -> p c (b s)")[:, ci, o:o + 512]
                    nc.vector.tensor_scalar_add(dst, px, b_in[:, ci:ci + 1])
            # xw for intra r/z gates, biases folded (z preact negated on host)
            xwf = xw4.rearrange("p g b s -> p g (b s)")
            for g in range(4):
                bias = b_r[:, g:g + 1] if g < 2 else nb_z[:, g - 2:g - 1]
                for j in range(2):
                    o = j * 512
                    px = psg.tile([128, 512], F32, tag="nn")
                    nc.tensor.matmul(px, wihT[:, g * 128:(g + 1) * 128],
                                     xflat[:, o:o + 512], start=True, stop=True)
                    nc.scalar.activation(xwf[:, g, o:o + 512], px, AF.Identity,
                                         bias=bias)

        # ---------------- phase 2: scans ----------------
        def inter_tile(t, o, w):
            h = h_inter
            freeze = (o + w) > MINACT[t]
            rz = psg.tile([128, 2, NT], F32, tag="rz")
            zz = psg.tile([128, 2, NT], F32, tag="zz")
            nn = psg.tile([128, 2, NT], F32, tag="nn")
            xt = xin[:, OFF[t] + o: OFF[t] + o + w]

            def gate_bank(ps, g0, fz):
                insts = []
                last = None
                for ci in range(2):
                    g = g0 + ci
                    sl = slice(g * 128, (g + 1) * 128)
                    mm = nc.tensor.matmul(ps[:, ci, :w], wihT[:, sl], xt,
                                          start=(ci == 0), stop=False)
                    insts.append(mm)
                    nc.tensor.matmul(ps[:, ci, :w], whhT[0][:, sl], h[:, 0, o:o + w],
                                     start=False, stop=False)
                    last = nc.tensor.matmul(ps[:, ci, :w], whhT[1][:, sl],
                                            h[:, 1, o:o + w],
                                            start=False, stop=(not fz) and ci == 1)
                    if fz:
                        last = nc.tensor.matmul(
                            ps[:, ci, :w], ones,
                            ind_all[:, OFF[t] + o: OFF[t] + o + w],
                            start=False, stop=(ci == 1))
                _coloc(insts)
                return last

            rz_last = gate_bank(rz, 0, False)
            zz_last = gate_bank(zz, 2, freeze)
            i0 = nc.tensor.matmul(nn[:, 0, :w], whhT[0][:, 512:640], h[:, 0, o:o + w],
                                  start=True, stop=False)
            nc.tensor.matmul(nn[:, 0, :w], whhT[1][:, 512:640], h[:, 1, o:o + w],
                             start=False, stop=False)
            i1 = nc.tensor.matmul(nn[:, 1, :w], whhT[0][:, 640:768], h[:, 0, o:o + w],
                                  start=False, stop=False)
            nn_last = nc.tensor.matmul(nn[:, 1, :w], whhT[1][:, 640:768],
                                       h[:, 1, o:o + w], start=False, stop=True)
            _coloc([i0, i1])

            r_sb = sb3.tile([128, 2, NT], BF16, tag="r_sb")
            zc_sb = sb3.tile([128, 2, NT], BF16, tag="zc_sb")
            t1_sb = sb3.tile([128, 2, NT], BF16, tag="t1_sb")
            u_sb = sb3.tile([128, 2, NT], BF16, tag="u_sb")
            n_sb = sb3.tile([128, 2, NT], BF16, tag="n_sb")
            d_sb = sb3.tile([128, 2, NT], BF16, tag="d_sb")
            f_sb = sb3.tile([128, 2, NT], BF16, tag="f_sb")
            for ci in range(2):
                _after(nc.scalar.activation(r_sb[:, ci, :w], rz[:, ci, :w], AF.Sigmoid,
                                            bias=b_r[:, ci:ci + 1]), rz_last)
                _after(nc.scalar.activation(zc_sb[:, ci, :w], zz[:, ci, :w], AF.Sigmoid,
                                            bias=nb_z[:, ci:ci + 1]), zz_last)
                _after(nc.vector.scalar_tensor_tensor(
                    t1_sb[:, ci, :w], nn[:, ci, :w], b_hn[:, ci:ci + 1], r_sb[:, ci, :w],
                    op0=ALU.add, op1=ALU.mult), nn_last)
            nc.vector.tensor_add(u_sb[:, :, :w], t1_sb[:, :, :w],
                                 xn_all[:, :, OFF[t] + o: OFF[t] + o + w])
            nc.scalar.activation(n_sb[:, :, :w], u_sb[:, :, :w], AF.Tanh)
            hsl = h[:, :, o:o + w]
            nc.gpsimd.tensor_sub(d_sb[:, :, :w], hsl, n_sb[:, :, :w])
            nc.gpsimd.tensor_mul(f_sb[:, :, :w], zc_sb[:, :, :w], d_sb[:, :, :w])
            nc.vector.tensor_sub(hsl, hsl, f_sb[:, :, :w])

        def intra_step(s):
            hprev = zeros16 if s == 0 else hT_all[:, :, :, s - 1]
            ia = psi.tile([128, 4, B], F32, tag="ia")
            ib = psi.tile([128, 2, B], F32, tag="ib")
            # xw (input proj + bias, z-negated) injected via one identity matmul;
            # only the 8 h-dependent matmuls sit on the step's critical path.
            id_mm = nc.tensor.matmul(ia.rearrange("p g b -> p (g b)"), id128e,
                                     xw4[:, :, :, s].rearrange("p g b -> p (g b)"),
                                     start=True, stop=False)
            insts = [id_mm]
            ia_last = None
            for g in range(4):
                sl = slice(g * 128, (g + 1) * 128)
                mm = nc.tensor.matmul(ia[:, g, :], whhT[0][:, sl], hprev[:, 0, :],
                                      start=False, stop=False)
                insts.append(mm)
                ia_last = nc.tensor.matmul(ia[:, g, :], whhT[1][:, sl], hprev[:, 1, :],
                                           start=False, stop=(g == 3))
            _coloc(insts)
            insts = []
            ib_last = None
            for ci in range(2):
                sl = slice(512 + ci * 128, 512 + (ci + 1) * 128)
                mm = nc.tensor.matmul(ib[:, ci, :], whhT[0][:, sl], hprev[:, 0, :],
                                      start=(ci == 0), stop=False)
                insts.append(mm)
                ib_last = nc.tensor.matmul(ib[:, ci, :], whhT[1][:, sl], hprev[:, 1, :],
                                           start=False, stop=(ci == 1))
            _coloc(insts)

            rz4 = sb2.tile([128, 4, B], BF16, tag="irz4")
            t1_sb = sb2.tile([128, 2, B], BF16, tag="it1_sb")
            u_sb = sb2.tile([128, 2, B], BF16, tag="iu_sb")
            n_sb = sb2.tile([128, 2, B], BF16, tag="in_sb")
            d_sb = sb2.tile([128, 2, B], BF16, tag="id_sb")
            f_sb = sb2.tile([128, 2, B], BF16, tag="if_sb")
            _after(nc.scalar.activation(rz4, ia, AF.Sigmoid), ia_last)
            for ci in range(2):
                _after(nc.vector.scalar_tensor_tensor(
                    t1_sb[:, ci, :], ib[:, ci, :], b_hn[:, ci:ci + 1], rz4[:, ci, :],
                    op0=ALU.add, op1=ALU.mult), ib_last)
            nc.vector.tensor_add(u_sb, t1_sb, xn_intra[:, :, :, s])
            nc.scalar.activation(n_sb, u_sb, AF.Tanh)
            nc.vector.tensor_sub(d_sb, hprev, n_sb)
            nc.vector.tensor_mul(f_sb, rz4[:, 2:4, :], d_sb)
            nc.vector.tensor_sub(hT_all[:, :, :, s], hprev, f_sb)

        # ---------------- interleaved attention pieces ----------------
        k_sb = singles.tile([128, NST, 256], BF16, tag="k_sb")
        v_sb = singles.tile([128, NST, 256], BF16, tag="v_sb")
        qa_sb = singles.tile([128, 2, 128], BF16, tag="qa_sb")
        ka_sb = singles.tile([128, 2, 128], BF16, tag="ka_sb")
        paT_all = singles.tile([S, BPC, 2, S], BF16, tag="paT_all")

        hflat = hT_all.rearrange("p c b s -> p c (b s)")   # [128, 2, 1024]
        hown = [hflat[:, ci, 0:NTOK] for ci in range(2)]    # [128, 128] each
        xflat_i = xintra.rearrange("d b s -> d (b s)")
        xp_own = xflat_i[0:127, 0:NTOK]                     # [127, 128]

        def proj_psx(lhs_chunks, rhs_tiles, bias_tile, out_sb):
            p = psg.tile([128, 2, NT], F32, tag="nn", name="p")[:, 0, :]
            first = True
            for (lt, rt) in zip(lhs_chunks, rhs_tiles):
                nc.tensor.matmul(p, lt, rt, start=first, stop=False)
                first = False
            nc.tensor.matmul(p, ones, bias_tile, start=False, stop=True)
            nc.vector.tensor_copy(out_sb, p)

        def kv_tile(s_):
            cols = slice(s_ * 128, (s_ + 1) * 128)
            proj_psx([h_inter[:, 0, cols], h_inter[:, 1, cols], rT[0:127, cols]],
                     [W_["ikw0"], W_["ikw1"], W_["ikwx"]], W_["ikb"], k_sb[:, s_, :])
            proj_psx([h_inter[:, 0, cols], h_inter[:, 1, cols], rT[:, cols]],
                     [W_["ivw0"], W_["ivw1"], W_["ivwx"]], W_["ivb"], v_sb[:, s_, :])

        def qaka():
            for wn, ob, bias in (("aqw", qa_sb, aqb), ("akw", ka_sb, akb)):
                ps = psg.tile([128, 2, NT], F32, tag="nn", name="ps")[:, :, 0:128]
                insts = []
                for ci in range(2):
                    mm = nc.tensor.matmul(ps[:, ci, :],
                                          W_[wn][:, ci * 128:(ci + 1) * 128],
                                          xp_own, start=(ci == 0), stop=(ci == 1))
                    insts.append(mm)
                _coloc(insts)
                for ci in range(2):
                    _after(nc.scalar.activation(ob[:, ci, :], ps[:, ci, :],
                                                AF.Identity, bias=bias[:, ci:ci + 1]),
                           insts[-1])

        def intra_chain(bl, hh):
            sca = psg.tile([128, 2, NT], F32, tag="nn", name="sca")[0:S, 0, 0:S]
            mm = nc.tensor.matmul(sca, qa_sb[:, hh, bl * S:(bl + 1) * S],
                                  ka_sb[:, hh, bl * S:(bl + 1) * S],
                                  start=True, stop=True)
            ms = sb3.tile([S, S], BF16, tag="ms")
            nc.vector.tensor_add(ms, sca, W_["cmask"])
            ex = sb3.tile([S, S], BF16, tag="ex")
            nc.scalar.activation(ex, ms, AF.Exp)
            rs = sb3.tile([S, 1], F32, tag="rs")
            nc.vector.tensor_reduce(rs, ex, axis=AX.X, op=ALU.add)
            ri = sb3.tile([S, 1], F32, tag="ri")
            nc.vector.reciprocal(ri, rs)
            pa = sb3.tile([S, S], BF16, tag="pa")
            nc.vector.tensor_scalar_mul(pa, ex, ri)
            ptp = psg.tile([S, S], BF16, tag="nn", name="ptp")
            nc.tensor.transpose(ptp, pa, W_["id128"][0:S, 0:S])
            nc.vector.tensor_copy(paT_all[:, bl, hh, :], ptp)

        # k/v tiles become final when the active width drops below their cols
        kv_after_step = {}
        for s_ in range(NST):
            ready = max((t for t in range(L) if W[t] > s_ * 128), default=-1)
            kv_after_step.setdefault(ready, []).append(s_)

        # ---------------- emission schedule ----------------
        xn_intra_all()
        XN_LEAD = 6
        for t in range(XN_LEAD):
            xn_inter_step(t)

        if KLEVEL == 1:
            ob = sb2.tile([128, 256], F32, tag="out_sb", name="ob")
            nc.vector.tensor_copy(ob, xn_all[:, 0, 0:256])
            nc.sync.dma_start(out=d_out.ap(), in_=ob)
            gru_psx.__exit__(None, None, None)
            gru_psi.__exit__(None, None, None)
            gru_ps.__exit__(None, None, None)
            return

        inter_iters = [(t, o, w) for t in range(L) for (o, w) in _tiles_of(W[t])]
        NT_TOT = len(inter_iters)
        emitted = 0
        xn_done = XN_LEAD
        kv_q = []
        kv_done = set()
        for i in range(S):
            intra_step(i)
            if i == 40:
                qaka()
            if i == 54:
                for bl in range(BPC):
                    for hh in range(2):
                        intra_chain(bl, hh)
            # drain ready k/v projections into the thin late-scan window
            if i >= 44 and kv_q:
                kv_tile(kv_q.pop(0))
            while xn_done < L and xn_done < XN_LEAD + (i * (L - XN_LEAD)) // 45:
                xn_inter_step(xn_done)
                xn_done += 1
            target = min(NT_TOT, ((i + 1) * NT_TOT) // S)
            while emitted < target:
                t, o, w = inter_iters[emitted]
                assert t < max(xn_done, XN_LEAD)
                inter_tile(t, o, w)
                emitted += 1
                last_of_step = (emitted == NT_TOT or inter_iters[emitted][0] != t)
                if last_of_step:
                    for s_ in kv_after_step.get(t, []):
                        kv_q.append(s_)
                        kv_done.add(s_)
        his_last = h_inter
        for s_ in kv_q + [s_ for s_ in range(NST) if s_ not in kv_done]:
            kv_tile(s_)
        gru_psi.__exit__(None, None, None)
        gru_ps.__exit__(None, None, None)

        if KLEVEL == 2:
            ob = sb2.tile([128, 256], F32, tag="out_sb", name="ob")
            nc.vector.tensor_copy(ob[:, 0:128], his_last[:, 0, 0:128])
            nc.vector.tensor_copy(ob[:, 128:256], hflat[:, 0, 0:128])
            nc.sync.dma_start(out=d_out.ap(), in_=ob)
            return

        # ---------------- phase 3: attention + fused final ----------------
        psa = ctx.enter_context(tc.tile_pool(name="psa", bufs=2, space="PSUM"))
        psb = ctx.enter_context(tc.tile_pool(name="psb", bufs=2, space="PSUM"))
        psf = ctx.enter_context(tc.tile_pool(name="psf", bufs=1, space="PSUM"))

        def proj(lhs_chunks, rhs_tiles, bias_tile, m_parts=128):
            p = psa.tile([m_parts, 256], F32, tag="proj")
            first = True
            for (lt, rt) in zip(lhs_chunks, rhs_tiles):
                nc.tensor.matmul(p, lt, rt, start=first, stop=False)
                first = False
            nc.tensor.matmul(p, ones[:, 0:m_parts], bias_tile, start=False, stop=True)
            return p

        q_ps = proj([hown[0], hown[1], xp_own],
                    [W_["iqw0"], W_["iqw1"], W_["iqwx"]], W_["iqb"])
        q_sb = sb2.tile([128, 256], BF16, tag="q_sb")
        nc.scalar.copy(q_sb, q_ps)

        # sorted-frame attention: q permute + scores + exp per 128-seq tile
        e_sb = singles.tile([128, NST, 2], BF16, tag="e_sb")
        e32 = singles.tile([128, NST, 2], F32, tag="e32")
        for s_ in range(NST):
            cols = slice(s_ * 128, (s_ + 1) * 128)
            qpp = psa.tile([128, 256], F32, tag="proj")
            nc.tensor.matmul(qpp, W_["Pq"][:, cols], q_sb, start=True, stop=True)
            qp_sb = sb3.tile([128, 256], BF16, tag="qp_sb")
            nc.scalar.copy(qp_sb, qpp)
            scratch = sb3.tile([128, 2, 128], BF16, tag="ttr_scratch")
            nc.vector.tensor_mul(scratch,
                                 qp_sb.rearrange("p (c n) -> p c n", c=2),
                                 k_sb[:, s_, :].rearrange("p (c n) -> p c n", c=2))
            sc = sb3.tile([128, 2, 1], F32, tag="sc")
            nc.vector.tensor_reduce(sc, scratch, axis=AX.X, op=ALU.add)
            nc.scalar.activation(e32[:, s_, :].rearrange("p (c one) -> p c one", c=2),
                                 sc, AF.Exp)
            nc.vector.tensor_copy(e_sb[:, s_, :], e32[:, s_, :])

        # esum per token + weighted values, co-located in one PSUM bank:
        # esum at cols 256:258, o at cols 0:256; single start=True clears bank.
        acc = psf.tile([128, 512], F32, tag="acc")
        esum_ps = acc[:, 256:258]
        o_ps = acc[:, 0:256]
        acc_insts = []
        for s_ in range(NST):
            cols = slice(s_ * 128, (s_ + 1) * 128)
            acc_insts.append(nc.tensor.matmul(
                esum_ps, W_["Pi"][:, cols], e_sb[:, s_, :],
                start=(s_ == 0), stop=False))
        ow_last = None
        for s_ in range(NST):
            cols = slice(s_ * 128, (s_ + 1) * 128)
            vw = sb3.tile([128, 256], BF16, tag="vw")
            for hh in range(2):
                hs = slice(hh * 128, (hh + 1) * 128)
                nc.vector.tensor_scalar_mul(vw[:, hs], v_sb[:, s_, hs],
                                            e32[:, s_, hh:hh + 1])
            ow_last = nc.tensor.matmul(o_ps, W_["Pi"][:, cols], vw,
                                       start=False, stop=(s_ == NST - 1))
            acc_insts.append(ow_last)
        _coloc(acc_insts)
        einv = sb2.tile([128, 2], F32, tag="einv")
        _after(nc.vector.reciprocal(einv, esum_ps), ow_last)
        o_i = sb2.tile([128, 256], BF16, tag="o_i")
        for hh in range(2):
            hs = slice(hh * 128, (hh + 1) * 128)
            _after(nc.vector.tensor_scalar_mul(o_i[:, hs], o_ps[:, hs],
                                               einv[:, hh:hh + 1]), ow_last)

        if KLEVEL == 27:
            ob = sb2.tile([128, 256], F32, tag="out_sb", name="ob")
            nc.vector.tensor_copy(ob, o_i)
            nc.sync.dma_start(out=d_out.ap(), in_=ob)
            return

        oiT = sb2.tile([128, 2, 128], BF16, tag="oiT")
        for ci in range(2):
            tp = psb.tile([128, 128], BF16, tag="tp", name="tp")
            nc.tensor.transpose(tp, o_i[:, ci * 128:(ci + 1) * 128], W_["id128"])
            nc.vector.tensor_copy(oiT[:, ci, :], tp)

        va_sb = []
        for bl in range(BPC):
            vp = proj([hT_all[:, 0, bl, :], hT_all[:, 1, bl, :], xlast[:, bl, :]],
                      [W_["avw0"], W_["avw1"], W_["avwx"]], W_["avb"], m_parts=S)
            vb = sb2.tile([S, 256], BF16, tag="va_sb")
            nc.scalar.copy(vb, vp)
            va_sb.append(vb)

        oaT = sb2.tile([128, 2, 128], BF16, tag="oaT")
        for bl in range(BPC):
            for hh in range(2):
                op = psb.tile([128, S], F32, tag="tp")
                nc.tensor.matmul(op, va_sb[bl][:, hh * 128:(hh + 1) * 128],
                                 paT_all[:, bl, hh, :], start=True, stop=True)
                nc.vector.tensor_copy(oaT[:, hh, bl * S:(bl + 1) * S], op)

        if KLEVEL == 29:
            ob = sb2.tile([128, 256], F32, tag="out_sb", name="ob")
            nc.vector.tensor_copy(ob[:, 0:128], oaT[:, 0, :])
            nc.vector.tensor_copy(ob[0:64, 128:256], va_sb[0][:, 0:128])
            nc.sync.dma_start(out=d_out.ap(), in_=ob[:, :])
            return

        # fused final projection
        fo = psf.tile([128, 512], F32, tag="acc", name="fo")[:, 0:256]
        nc.tensor.matmul(fo, oiT[:, 0, :], W_["AiT0"], start=True, stop=False)
        nc.tensor.matmul(fo, oiT[:, 1, :], W_["AiT1"], start=False, stop=False)
        nc.tensor.matmul(fo, oaT[:, 0, :], W_["AaT0"], start=False, stop=False)
        nc.tensor.matmul(fo, oaT[:, 1, :], W_["AaT1"], start=False, stop=False)
        nc.tensor.matmul(fo, hown[0], W_["LhT0"], start=False, stop=False)
        nc.tensor.matmul(fo, hown[1], W_["LhT1"], start=False, stop=False)
        nc.tensor.matmul(fo, xp_own, W_["LxT"], start=False, stop=False)
        nc.tensor.matmul(fo, ones, W_["btot"], start=False, stop=True)
        out_sb = sb2.tile([128, 256], F32, tag="out_sb")
        nc.vector.tensor_copy(out_sb, fo)
        nc.sync.dma_start(out=d_out.ap(), in_=out_sb)


def _build(W, MINACT):
    OFF = [0]
    for t in range(L - 1):
        OFF.append(OFF[t] + W[t])
    TOTAL = OFF[-1] + W[-1]

    nc = bacc.Bacc("TRN2", target_bir_lowering=False, debug=False)
    di = {}

    def inp(name, shape, dt=BF16):
        di[name] = nc.dram_tensor(name, list(shape), dt, kind="ExternalInput")

    inp("xinter", [128, TOTAL])
    inp("xintra", [128, B, S])
    inp("xlast", [1, B, S])
    inp("rT", [128, NSEQ])
    inp("indr", [1, TOTAL])
    inp("wihT", [128, 768])
    inp("whh0T", [128, 768])
    inp("whh1T", [128, 768])
    inp("blobB", [128, BLOBW])
    inp("blobC", [1, 256 * len(_BLOBC_NAMES)])
    inp("blobD", [128, 2 * len(_BLOBD_NAMES)], F32)
    inp("id128e", [128, 128])

    d_out = nc.dram_tensor("out", [NTOK, 256], F32, kind="ExternalOutput")

    with tile.TileContext(nc) as tc:
        _emit(nc, tc, di, d_out, W, OFF, MINACT)
    nc.compile()
    return nc


# ----------------------------------------------------------------------------
# host-side prep
# ----------------------------------------------------------------------------

def _plan(inter_len):
    """Per-core length-sort plan + shared compile-time widths."""
    lens5 = np.asarray(inter_len, np.int64).reshape(B, S, R)
    orders, lens_sorted = [], []
    act = np.zeros((NCORES, L), np.int64)
    for c in range(NCORES):
        lens = lens5[[2 * c, 2 * c + 1]].reshape(NSEQ)
        order = np.argsort(-lens, kind="stable")
        ls = lens[order]
        orders.append(order)
        lens_sorted.append(ls)
        for t in range(L):
            act[c, t] = int((ls > t).sum())
    W = [min(NSEQ, int(-32 * (-(act[:, t].max()) // 32))) for t in range(L)]
    W = [max(32, w) for w in W]
    for t in range(1, L):
        W[t] = min(W[t], W[t - 1])
    MINACT = [int(act[:, t].min()) for t in range(L)]
    OFF = [0]
    for t in range(L - 1):
        OFF.append(OFF[t] + W[t])
    return orders, lens_sorted, W, MINACT, OFF


def prep_in_maps(inputs):
    inp = {k: np.asarray(v) for k, v in inputs.items()}
    w_ih = f32c(inp["w_ih"])
    w_hh = f32c(inp["w_hh"])
    b_ih = f32c(inp["b_ih"])
    b_hh = f32c(inp["b_hh"])
    b_rz = b_ih[:2 * H] + b_hh[:2 * H]
    sq = np.sqrt(128.0)

    e = np.exp(f32c(inp["wr"])[0, 0] - f32c(inp["wr"])[0, 0].max())
    w01 = e / e.sum()
    ln_w = f32c(inp["ln_w"])
    L_v, L_h, L_x = ln_w[:, :H], ln_w[:, H:2 * H], ln_w[:, 2 * H:]
    Ai = w01[0] * (L_v @ f32c(inp["io_w"]))
    Aa = w01[1] * (L_v @ f32c(inp["ao_w"]))
    btot = f32c(inp["ln_b"]) + L_v @ (w01[0] * f32c(inp["io_b"]) + w01[1] * f32c(inp["ao_b"]))

    iq_w = f32c(inp["iq_w"]) / sq
    iq_b = f32c(inp["iq_b"]) / sq
    aq_w = f32c(inp["aq_w"]) / sq
    aq_b = f32c(inp["aq_b"]) / sq

    def chunks2(m):  # [128,2] fp32 per-partition chunk tiles
        return f32c(np.stack([m[:128], m[128:256]], axis=1))

    orders, lens_sorted, W, MINACT, OFF = _plan(inp["inter_len"])
    TOTAL = OFF[-1] + W[-1]

    x_bs = f32c(inp["intra_x"])                     # [B,S,D]
    his5 = f32c(inp["inter_his"]).reshape(B, S, R, L, D)
    r5 = f32c(inp["inter_r"]).reshape(B, S, R, D)

    # shared weight blobs
    bw = {
        "iqw0": iq_w.T[0:128], "iqw1": iq_w.T[128:256], "iqwx": iq_w.T[256:383],
        "ikw0": inp["ik_w"].T[0:128], "ikw1": inp["ik_w"].T[128:256],
        "ikwx": inp["ik_w"].T[256:383],
        "ivw0": inp["iv_w"].T[0:128], "ivw1": inp["iv_w"].T[128:256],
        "ivwx": inp["iv_w"].T[256:384],
        "aqw": aq_w.T, "akw": f32c(inp["ak_w"]).T,
        "avw0": inp["av_w"].T[0:128], "avw1": inp["av_w"].T[128:256],
        "AiT0": Ai.T[0:128], "AiT1": Ai.T[128:256],
        "AaT0": Aa.T[0:128], "AaT1": Aa.T[128:256],
        "LhT0": L_h.T[0:128], "LhT1": L_h.T[128:256], "LxT": L_x.T,
        "id128": np.eye(128, dtype=np.float32),
        "cmask": np.where(np.tril(np.ones((S, S), bool)), 0.0, -BIG),
    }
    blobC = np.zeros((1, 256 * len(_BLOBC_NAMES)), np.float32)
    bc = {
        "iqb": iq_b, "ikb": f32c(inp["ik_b"]), "ivb": f32c(inp["iv_b"]),
        "avwx": f32c(inp["av_w"]).T[256], "avb": f32c(inp["av_b"]), "btot": btot,
    }
    for i, nm in enumerate(_BLOBC_NAMES):
        blobC[0, 256 * i:256 * i + len(bc[nm])] = bc[nm]
    blobD = np.zeros((128, 2 * len(_BLOBD_NAMES)), np.float32)
    bd = {
        "b_r": chunks2(b_rz[:H]), "nb_z": chunks2(-b_rz[H:]),
        "b_in": chunks2(b_ih[2 * H:]), "b_hn": chunks2(b_hh[2 * H:]),
        "aqb": chunks2(aq_b), "akb": chunks2(f32c(inp["ak_b"])),
    }
    for i, nm in enumerate(_BLOBD_NAMES):
        blobD[:, 2 * i:2 * i + 2] = bd[nm]

    # negate the z-gate (columns 256:512 of the transposed weights) so the
    # z preactivation accumulates negated and sigmoid needs no scale flip
    wihT_h = w_ih.T.copy()
    wihT_h[:, 256:512] *= -1.0
    whhT_h = w_hh.T.copy()
    whhT_h[:, 256:512] *= -1.0
    shared = dict(
        wihT=bfc(wihT_h),
        whh0T=bfc(whhT_h[0:128]),
        whh1T=bfc(whhT_h[128:256]),
        blobC=bfc(blobC),
        blobD=f32c(blobD),
        id128e=bfc(np.eye(128, dtype=np.float32)),
    )

    in_maps = []
    for c in range(NCORES):
        bsel = [2 * c, 2 * c + 1]
        order = orders[c]
        ls = lens_sorted[c]
        # inter: sorted seq order; orig col order is ((bl,s),r)
        his_cols = his5[bsel].reshape(NSEQ, L, D)[order]    # [NSEQ, L, D]
        xint = np.zeros((D, TOTAL), np.float32)
        ind = np.zeros((1, TOTAL), np.float32)
        for t in range(L):
            o, w = OFF[t], W[t]
            xint[:, o:o + w] = his_cols[:w, t, :].T
            ind[0, o:o + w] = -BIG * (t >= ls[:w])
        rTc = r5[bsel].reshape(NSEQ, D)[order].T            # [D, NSEQ]
        # permutation matrices: tok_of[p] for sorted position p
        tok_of = order // R
        Pq = np.zeros((128, NSEQ), np.float32)
        Pi = np.zeros((128, NSEQ), np.float32)
        for s_ in range(NST):
            for pl in range(128):
                tok = tok_of[s_ * 128 + pl]
                Pq[tok, s_ * 128 + pl] = 1.0
                Pi[pl, s_ * 128 + tok] = 1.0
        blobB = np.zeros((128, BLOBW), np.float32)
        for nm, (o_, w_) in _BLOB_OFF.items():
            src = {"Pq": Pq, "Pi": Pi}.get(nm)
            if src is None:
                src = bw[nm]
            blobB[0:src.shape[0], o_:o_ + src.shape[1]] = src
        # intra: batches rotated so own batches are 0..1; (d, b, s) layout
        rolled = np.roll(x_bs, -2 * c, axis=0)
        xia = rolled.transpose(2, 0, 1)             # [D, B, S]
        m = dict(shared)
        m.update(
            xinter=bfc(xint),
            xintra=bfc(xia),
            xlast=bfc(xia[127:128]),
            rT=bfc(rTc),
            indr=bfc(ind),
            blobB=bfc(blobB),
        )
        in_maps.append(m)
    return in_maps, W, MINACT


def assemble(core_outs):
    o = np.stack([np.asarray(co, np.float32) for co in core_outs])  # [8,128,256]
    return np.ascontiguousarray(o.reshape(B * S, 256))


_CACHE = {}


def kernel(**inputs) -> np.ndarray:
    in_maps, W, MINACT = prep_in_maps(inputs)
    key = (tuple(W), tuple(MINACT))
    if _CACHE.get("key") != key:
        _CACHE["nc"] = _build(W, MINACT)
        _CACHE["key"] = key
    nc = _CACHE["nc"]
    res = run_bass_kernel_spmd(nc, in_maps, core_ids=list(range(NCORES)))
    return assemble([r["out"] for r in res.results])


# revision 37
# speedup vs baseline: 1.2478x; 1.0311x over previous
"""Trainium2 Bass kernel for nn_CoKT (dual GRU + cross/causal attention + fused linear).

Self-contained: builds an 8-core SPMD Tile kernel, shards tokens (B*S) across
cores (2 batches/core), replicates weights, runs via run_bass_kernel_spmd,
reassembles the full [1024, 256] fp32 output.

Per-core design (128 own tokens, core-local order (bl, s)):
- inter GRU: 768 seqs x 24 steps, seqs GLOBALLY SORTED by length (descending)
  on host; step t only processes the active prefix W(t) = max-over-cores
  active count rounded to 32 (z-freeze +BIG trick protects the padded tail of
  each boundary tile). Cuts seq-step work ~1.8x vs dense. h updated in place.
- attention runs in the sorted frame: q is permuted per 128-seq tile with
  host-built permutation matmuls; softmax normalization and the output
  accumulation go back to token frame via indicator matmuls.
- k/v projections of finished seq-tiles and the intra-attention score/softmax
  chains are interleaved INTO the scan (they only depend on frozen h columns
  resp. xintra), shrinking the serial tail.
- intra GRU: batch 16 x 64 steps, replicated on every core (weight-load bound
  either way); host rotates batches so own 2 batches are columns 0..1.
- xinter/ind fully SBUF-resident (one DMA each); attention weights packed
  into one blob DMA'd mid-scan.
"""
import sys
if "/opt/trn_rl_repo" not in sys.path:
    sys.path.insert(0, "/opt/trn_rl_repo")

import numpy as np
import ml_dtypes

import concourse.bacc as bacc
import concourse.mybir as mybir
import concourse.tile as tile
from concourse.tile import add_dep_helper
from concourse.bass_utils import run_bass_kernel_spmd

F32 = mybir.dt.float32
BF16 = mybir.dt.bfloat16
AF = mybir.ActivationFunctionType
ALU = mybir.AluOpType
AX = mybir.AxisListType

B, S, R, L, D, H = 16, 64, 6, 24, 128, 256
NCORES = 8
BPC = B // NCORES            # 2 batches per core
NTOK = S * BPC               # 128 own tokens
NSEQ = NTOK * R              # 768 inter sequences per core
NT = 256                     # inter token-tile width
BIG = 30000.0
NST = NSEQ // 128            # 6 seq-tiles of 128 in the attention phase

bfc = lambda x: np.ascontiguousarray(np.asarray(x, np.float32).astype(ml_dtypes.bfloat16))
f32c = lambda x: np.ascontiguousarray(np.asarray(x, np.float32))

# attention-weight blob layout: name -> (col offset, width); all 128 rows
# (127-row tensors sit in rows 0:127, cmask in rows 0:64).
_BLOB_NAMES = [
    ("iqw0", 256), ("iqw1", 256), ("ikw0", 256), ("ikw1", 256),
    ("ivw0", 256), ("ivw1", 256), ("ivwx", 256), ("avw0", 256), ("avw1", 256),
    ("AiT0", 256), ("AiT1", 256), ("AaT0", 256), ("AaT1", 256),
    ("LhT0", 256), ("LhT1", 256),
    ("iqwx", 256), ("ikwx", 256), ("aqw", 256), ("akw", 256), ("LxT", 256),
    ("id128", 128), ("Pq", NSEQ), ("Pi", NSEQ), ("cmask", S),
]
_BLOB_OFF = {}
_off = 0
for _nm, _w in _BLOB_NAMES:
    _BLOB_OFF[_nm] = (_off, _w)
    _off += _w
BLOBW = _off
_ROWS127 = {"iqwx", "ikwx", "aqw", "akw", "LxT"}

_BLOBC_NAMES = ["iqb", "ikb", "ivb", "avwx", "avb", "btot"]
_BLOBD_NAMES = ["b_r", "nb_z", "b_in", "b_hn", "aqb", "akb"]


def _tiles_of(w):
    out = []
    o = 0
    while o < w:
        out.append((o, min(NT, w - o)))
        o += NT
    return out


# ----------------------------------------------------------------------------
# device program
# ----------------------------------------------------------------------------

def _coloc(insts):
    first = insts[0]
    for x in insts[1:]:
        add_dep_helper(x.ins, first.ins, sync=True, reason="psum coloc order")


def _after(consumer, last_mm):
    """PSUM banks are single-port: a reader of one co-located half must wait
    until the PE is done with the WHOLE bank (fatal collision otherwise)."""
    add_dep_helper(consumer.ins, last_mm.ins, sync=True, reason="bank read-after-all-mm")


def _emit(nc, tc, di, d_out, W, OFF, MINACT):
    import os
    KLEVEL = int(os.environ.get("KLEVEL", "3"))
    import contextlib
    ctx = contextlib.ExitStack()
    TOTAL = OFF[-1] + W[-1]
    with ctx:
        singles = ctx.enter_context(tc.tile_pool(name="singles", bufs=1))
        sb2 = ctx.enter_context(tc.tile_pool(name="work2", bufs=2))
        sb3 = ctx.enter_context(tc.tile_pool(name="work3", bufs=3))

        def load(name):
            d = di[name]
            t = singles.tile(list(d.shape), d.dtype, tag=name)
            nc.sync.dma_start(out=t, in_=d.ap())
            return t

        # early loads (scan-phase inputs)
        wihT = load("wihT")
        xintra = load("xintra")
        whhT = [load("whh0T"), load("whh1T")]
        blobD = load("blobD")
        id128e = load("id128e")
        xin = load("xinter")
        ind_all = load("indr")
        rT = load("rT")
        xlast = load("xlast")
        bD = {nm: blobD[:, 2 * i:2 * i + 2] for i, nm in enumerate(_BLOBD_NAMES)}
        b_r, nb_z, b_in, b_hn = bD["b_r"], bD["nb_z"], bD["b_in"], bD["b_hn"]
        aqb, akb = bD["aqb"], bD["akb"]

        # deferred loads (attention weights; DMA emitted here, lands mid-scan
        # behind the big xinter transfer on the sync queue)
        blobB = singles.tile([128, BLOBW], BF16, tag="blobB")
        nc.sync.dma_start(out=blobB, in_=di["blobB"].ap())
        blobC = singles.tile([1, 256 * len(_BLOBC_NAMES)], BF16, tag="blobC")
        nc.sync.dma_start(out=blobC, in_=di["blobC"].ap())
        W_ = {}
        for nm, (o_, w_) in _BLOB_OFF.items():
            rows = 127 if nm in _ROWS127 else (S if nm == "cmask" else 128)
            W_[nm] = blobB[0:rows, o_:o_ + w_]
        for i, nm in enumerate(_BLOBC_NAMES):
            W_[nm] = blobC[:, 256 * i:256 * (i + 1)]
        W_["id128"] = id128e

        ones = singles.tile([1, 128], BF16, tag="ones")
        nc.vector.memset(ones, 1.0)

        xn_all = singles.tile([128, 2, TOTAL], BF16, tag="xn_all")
        xn_intra = singles.tile([128, 2, B, S], BF16, tag="xn_intra")
        xw4 = singles.tile([128, 4, B, S], BF16, tag="xw4")
        hT_all = singles.tile([128, 2, B, S], BF16, tag="hT_all")
        zeros16 = singles.tile([128, 2, B], BF16, tag="zeros16")
        nc.vector.memset(zeros16, 0.0)
        h_inter = singles.tile([128, 2, NSEQ], BF16, tag="h_inter")
        nc.vector.memset(h_inter, 0.0)

        # GRU-phase psum pools: rz/zz/nn x2 (6 banks) + ia/ib (2) = 8; the
        # interleaved attention pieces borrow the psg "nn" tag's banks.
        gru_ps = tc.tile_pool(name="psg", bufs=2, space="PSUM")
        psg = gru_ps.__enter__()
        gru_psi = tc.tile_pool(name="psi", bufs=1, space="PSUM")
        psi = gru_psi.__enter__()

        # ---------------- phase 1 pieces: xn = w_ih_n @ x (+b_in via evac) ----
        def xn_inter_step(t):
            for j, (o, w) in enumerate(_tiles_of(W[t])):
                xt = xin[:, OFF[t] + o: OFF[t] + o + w]
                px = psg.tile([128, 2, NT], F32, tag=("rz", "zz", "nn")[(t + j) % 3],
                              name="px")
                m0 = nc.tensor.matmul(px[:, 0, :w], wihT[:, 512:640],
                                      xt, start=True, stop=False)
                m1 = nc.tensor.matmul(px[:, 1, :w], wihT[:, 640:768],
                                      xt, start=False, stop=True)
                _coloc([m0, m1])
                dst = xn_all[:, :, OFF[t] + o: OFF[t] + o + w]
                ev0 = nc.scalar.activation(dst[:, 0, :], px[:, 0, :w], AF.Identity,
                                           bias=b_in[:, 0:1])
                _after(ev0, m1)
                nc.vector.tensor_scalar_add(dst[:, 1, :], px[:, 1, :w], b_in[:, 1:2])

        def xn_intra_all():
            xflat = xintra.rearrange("d b s -> d (b s)")
            for j in range(2):
                o = j * 512
                for ci in range(2):
                    px = psg.tile([128, 512], F32, tag="nn")
                    nc.tensor.matmul(px, wihT[:, 512 + ci * 128: 640 + ci * 128],
                                     xflat[:, o:o + 512], start=True, stop=True)
                    dst = xn_intra.rearrange("p c b s -> p c (b s)")[:, ci, o:o + 512]
                    nc.vector.tensor_scalar_add(dst, px, b_in[:, ci:ci + 1])
            # xw for intra r/z gates, biases folded (z preact negated on host)
            xwf = xw4.rearrange("p g b s -> p g (b s)")
            for g in range(4):
                bias = b_r[:, g:g + 1] if g < 2 else nb_z[:, g - 2:g - 1]
                for j in range(2):
                    o = j * 512
                    px = psg.tile([128, 512], F32, tag="nn")
                    nc.tensor.matmul(px, wihT[:, g * 128:(g + 1) * 128],
                                     xflat[:, o:o + 512], start=True, stop=True)
                    nc.scalar.activation(xwf[:, g, o:o + 512], px, AF.Identity,
                                         bias=bias)

        # ---------------- phase 2: scans ----------------
        def inter_tile(t, o, w):
            h = h_inter
            freeze = (o + w) > MINACT[t]
            rz = psg.tile([128, 2, NT], F32, tag="rz")
            zz = psg.tile([128, 2, NT], F32, tag="zz")
            nn = psg.tile([128, 2, NT], F32, tag="nn")
            xt = xin[:, OFF[t] + o: OFF[t] + o + w]

            def gate_bank(ps, g0, fz):
                insts = []
                last = None
                for ci in range(2):
                    g = g0 + ci
                    sl = slice(g * 128, (g + 1) * 128)
                    mm = nc.tensor.matmul(ps[:, ci, :w], wihT[:, sl], xt,
                                          start=(ci == 0), stop=False)
                    insts.append(mm)
                    nc.tensor.matmul(ps[:, ci, :w], whhT[0][:, sl], h[:, 0, o:o + w],
                                     start=False, stop=False)
                    last = nc.tensor.matmul(ps[:, ci, :w], whhT[1][:, sl],
                                            h[:, 1, o:o + w],
                                            start=False, stop=(not fz) and ci == 1)
                    if fz:
                        last = nc.tensor.matmul(
                            ps[:, ci, :w], ones,
                            ind_all[:, OFF[t] + o: OFF[t] + o + w],
                            start=False, stop=(ci == 1))
                _coloc(insts)
                return last

            rz_last = gate_bank(rz, 0, False)
            zz_last = gate_bank(zz, 2, freeze)
            i0 = nc.tensor.matmul(nn[:, 0, :w], whhT[0][:, 512:640], h[:, 0, o:o + w],
                                  start=True, stop=False)
            nc.tensor.matmul(nn[:, 0, :w], whhT[1][:, 512:640], h[:, 1, o:o + w],
                             start=False, stop=False)
            i1 = nc.tensor.matmul(nn[:, 1, :w], whhT[0][:, 640:768], h[:, 0, o:o + w],
                                  start=False, stop=False)
            nn_last = nc.tensor.matmul(nn[:, 1, :w], whhT[1][:, 640:768],
                                       h[:, 1, o:o + w], start=False, stop=True)
            _coloc([i0, i1])

            r_sb = sb3.tile([128, 2, NT], BF16, tag="r_sb")
            zc_sb = sb3.tile([128, 2, NT], BF16, tag="zc_sb")
            t1_sb = sb3.tile([128, 2, NT], BF16, tag="t1_sb")
            u_sb = sb3.tile([128, 2, NT], BF16, tag="u_sb")
            n_sb = sb3.tile([128, 2, NT], BF16, tag="n_sb")
            d_sb = sb3.tile([128, 2, NT], BF16, tag="d_sb")
            f_sb = sb3.tile([128, 2, NT], BF16, tag="f_sb")
            for ci in range(2):
                _after(nc.scalar.activation(r_sb[:, ci, :w], rz[:, ci, :w], AF.Sigmoid,
                                            bias=b_r[:, ci:ci + 1]), rz_last)
                _after(nc.scalar.activation(zc_sb[:, ci, :w], zz[:, ci, :w], AF.Sigmoid,
                                            bias=nb_z[:, ci:ci + 1]), zz_last)
                _after(nc.vector.scalar_tensor_tensor(
                    t1_sb[:, ci, :w], nn[:, ci, :w], b_hn[:, ci:ci + 1], r_sb[:, ci, :w],
                    op0=ALU.add, op1=ALU.mult), nn_last)
            nc.vector.tensor_add(u_sb[:, :, :w], t1_sb[:, :, :w],
                                 xn_all[:, :, OFF[t] + o: OFF[t] + o + w])
            nc.scalar.activation(n_sb[:, :, :w], u_sb[:, :, :w], AF.Tanh)
            hsl = h[:, :, o:o + w]
            nc.gpsimd.tensor_sub(d_sb[:, :, :w], hsl, n_sb[:, :, :w])
            nc.gpsimd.tensor_mul(f_sb[:, :, :w], zc_sb[:, :, :w], d_sb[:, :, :w])
            nc.gpsimd.tensor_sub(hsl, hsl, f_sb[:, :, :w])

        def intra_step(s):
            hprev = zeros16 if s == 0 else hT_all[:, :, :, s - 1]
            ia = psi.tile([128, 4, B], F32, tag="ia")
            ib = psi.tile([128, 2, B], F32, tag="ib")
            # xw (input proj + bias, z-negated) injected via one identity matmul;
            # only the 8 h-dependent matmuls sit on the step's critical path.
            id_mm = nc.tensor.matmul(ia.rearrange("p g b -> p (g b)"), id128e,
                                     xw4[:, :, :, s].rearrange("p g b -> p (g b)"),
                                     start=True, stop=False)
            insts = [id_mm]
            ia_last = None
            for g in range(4):
                sl = slice(g * 128, (g + 1) * 128)
                mm = nc.tensor.matmul(ia[:, g, :], whhT[0][:, sl], hprev[:, 0, :],
                                      start=False, stop=False)
                insts.append(mm)
                ia_last = nc.tensor.matmul(ia[:, g, :], whhT[1][:, sl], hprev[:, 1, :],
                                           start=False, stop=(g == 3))
            _coloc(insts)
            insts = []
            ib_last = None
            for ci in range(2):
                sl = slice(512 + ci * 128, 512 + (ci + 1) * 128)
                mm = nc.tensor.matmul(ib[:, ci, :], whhT[0][:, sl], hprev[:, 0, :],
                                      start=(ci == 0), stop=False)
                insts.append(mm)
                ib_last = nc.tensor.matmul(ib[:, ci, :], whhT[1][:, sl], hprev[:, 1, :],
                                           start=False, stop=(ci == 1))
            _coloc(insts)

            rz4 = sb2.tile([128, 4, B], BF16, tag="irz4")
            t1_sb = sb2.tile([128, 2, B], BF16, tag="it1_sb")
            u_sb = sb2.tile([128, 2, B], BF16, tag="iu_sb")
            n_sb = sb2.tile([128, 2, B], BF16, tag="in_sb")
            d_sb = sb2.tile([128, 2, B], BF16, tag="id_sb")
            f_sb = sb2.tile([128, 2, B], BF16, tag="if_sb")
            _after(nc.scalar.activation(rz4, ia, AF.Sigmoid), ia_last)
            for ci in range(2):
                _after(nc.vector.scalar_tensor_tensor(
                    t1_sb[:, ci, :], ib[:, ci, :], b_hn[:, ci:ci + 1], rz4[:, ci, :],
                    op0=ALU.add, op1=ALU.mult), ib_last)
            nc.vector.tensor_add(u_sb, t1_sb, xn_intra[:, :, :, s])
            nc.scalar.activation(n_sb, u_sb, AF.Tanh)
            nc.vector.tensor_sub(d_sb, hprev, n_sb)
            nc.vector.tensor_mul(f_sb, rz4[:, 2:4, :], d_sb)
            nc.vector.tensor_sub(hT_all[:, :, :, s], hprev, f_sb)

        # ---------------- interleaved attention pieces ----------------
        k_sb = singles.tile([128, NST, 256], BF16, tag="k_sb")
        v_sb = singles.tile([128, NST, 256], BF16, tag="v_sb")
        qa_sb = singles.tile([128, 2, 128], BF16, tag="qa_sb")
        ka_sb = singles.tile([128, 2, 128], BF16, tag="ka_sb")
        ms_all = singles.tile([S, BPC, 2, S], BF16, tag="ms_all")

        hflat = hT_all.rearrange("p c b s -> p c (b s)")   # [128, 2, 1024]
        hown = [hflat[:, ci, 0:NTOK] for ci in range(2)]    # [128, 128] each
        xflat_i = xintra.rearrange("d b s -> d (b s)")
        xp_own = xflat_i[0:127, 0:NTOK]                     # [127, 128]

        def proj_psx(lhs_chunks, rhs_tiles, bias_tile, out_sb):
            p = psg.tile([128, 2, NT], F32, tag="nn", name="p")[:, 0, :]
            first = True
            for (lt, rt) in zip(lhs_chunks, rhs_tiles):
                nc.tensor.matmul(p, lt, rt, start=first, stop=False)
                first = False
            nc.tensor.matmul(p, ones, bias_tile, start=False, stop=True)
            nc.scalar.copy(out_sb, p)

        def kv_tile(s_):
            cols = slice(s_ * 128, (s_ + 1) * 128)
            proj_psx([h_inter[:, 0, cols], h_inter[:, 1, cols], rT[0:127, cols]],
                     [W_["ikw0"], W_["ikw1"], W_["ikwx"]], W_["ikb"], k_sb[:, s_, :])
            proj_psx([h_inter[:, 0, cols], h_inter[:, 1, cols], rT[:, cols]],
                     [W_["ivw0"], W_["ivw1"], W_["ivwx"]], W_["ivb"], v_sb[:, s_, :])

        def qaka():
            for wn, ob, bias in (("aqw", qa_sb, aqb), ("akw", ka_sb, akb)):
                ps = psg.tile([128, 2, NT], F32, tag="nn", name="ps")[:, :, 0:128]
                insts = []
                for ci in range(2):
                    mm = nc.tensor.matmul(ps[:, ci, :],
                                          W_[wn][:, ci * 128:(ci + 1) * 128],
                                          xp_own, start=(ci == 0), stop=(ci == 1))
                    insts.append(mm)
                _coloc(insts)
                for ci in range(2):
                    _after(nc.scalar.activation(ob[:, ci, :], ps[:, ci, :],
                                                AF.Identity, bias=bias[:, ci:ci + 1]),
                           insts[-1])

        def intra_chain(bl, hh):
            # in-scan part: scores + causal mask only (exp deferred to the tail
            # so the ACT engine's sig/tanh tables aren't thrashed mid-scan)
            sca = psg.tile([128, 2, NT], F32, tag="nn", name="sca")[0:S, 0, 0:S]
            nc.tensor.matmul(sca, qa_sb[:, hh, bl * S:(bl + 1) * S],
                             ka_sb[:, hh, bl * S:(bl + 1) * S],
                             start=True, stop=True)
            nc.vector.tensor_add(ms_all[:, bl, hh, :], sca, W_["cmask"])

        # k/v tiles become final when the active width drops below their cols
        kv_after_step = {}
        for s_ in range(NST):
            ready = max((t for t in range(L) if W[t] > s_ * 128), default=-1)
            kv_after_step.setdefault(ready, []).append(s_)

        # ---------------- emission schedule ----------------
        xn_intra_all()
        XN_LEAD = 6
        for t in range(XN_LEAD):
            xn_inter_step(t)

        if KLEVEL == 1:
            ob = sb2.tile([128, 256], F32, tag="out_sb", name="ob")
            nc.vector.tensor_copy(ob, xn_all[:, 0, 0:256])
            nc.sync.dma_start(out=d_out.ap(), in_=ob)
            gru_psx.__exit__(None, None, None)
            gru_psi.__exit__(None, None, None)
            gru_ps.__exit__(None, None, None)
            return

        inter_iters = [(t, o, w) for t in range(L) for (o, w) in _tiles_of(W[t])]
        NT_TOT = len(inter_iters)
        emitted = 0
        xn_done = XN_LEAD
        kv_q = []
        kv_done = set()
        for i in range(S):
            intra_step(i)
            if i == 40:
                qaka()
            if i == 54:
                for bl in range(BPC):
                    for hh in range(2):
                        intra_chain(bl, hh)
            # drain ready k/v projections into the thin late-scan window
            if i >= 44 and kv_q:
                kv_tile(kv_q.pop(0))
            while xn_done < L and xn_done < XN_LEAD + (i * (L - XN_LEAD)) // 45:
                xn_inter_step(xn_done)
                xn_done += 1
            target = min(NT_TOT, ((i + 1) * NT_TOT) // S)
            while emitted < target:
                t, o, w = inter_iters[emitted]
                assert t < max(xn_done, XN_LEAD)
                inter_tile(t, o, w)
                emitted += 1
                last_of_step = (emitted == NT_TOT or inter_iters[emitted][0] != t)
                if last_of_step:
                    for s_ in kv_after_step.get(t, []):
                        kv_q.append(s_)
                        kv_done.add(s_)
        his_last = h_inter
        for s_ in kv_q + [s_ for s_ in range(NST) if s_ not in kv_done]:
            kv_tile(s_)
        gru_psi.__exit__(None, None, None)
        gru_ps.__exit__(None, None, None)

        if KLEVEL == 2:
            ob = sb2.tile([128, 256], F32, tag="out_sb", name="ob")
            nc.vector.tensor_copy(ob[:, 0:128], his_last[:, 0, 0:128])
            nc.vector.tensor_copy(ob[:, 128:256], hflat[:, 0, 0:128])
            nc.sync.dma_start(out=d_out.ap(), in_=ob)
            return

        # ---------------- phase 3: attention + fused final ----------------
        psa = ctx.enter_context(tc.tile_pool(name="psa", bufs=2, space="PSUM"))
        psb = ctx.enter_context(tc.tile_pool(name="psb", bufs=2, space="PSUM"))
        psf = ctx.enter_context(tc.tile_pool(name="psf", bufs=1, space="PSUM"))

        def proj(lhs_chunks, rhs_tiles, bias_tile, m_parts=128):
            p = psa.tile([m_parts, 256], F32, tag="proj")
            first = True
            for (lt, rt) in zip(lhs_chunks, rhs_tiles):
                nc.tensor.matmul(p, lt, rt, start=first, stop=False)
                first = False
            nc.tensor.matmul(p, ones[:, 0:m_parts], bias_tile, start=False, stop=True)
            return p

        q_ps = proj([hown[0], hown[1], xp_own],
                    [W_["iqw0"], W_["iqw1"], W_["iqwx"]], W_["iqb"])
        q_sb = sb2.tile([128, 256], BF16, tag="q_sb")
        nc.scalar.copy(q_sb, q_ps)

        # sorted-frame attention: q permute + scores + exp per 128-seq tile
        e_sb = singles.tile([128, NST, 2], BF16, tag="e_sb")
        e32 = singles.tile([128, NST, 2], F32, tag="e32")
        for s_ in range(NST):
            cols = slice(s_ * 128, (s_ + 1) * 128)
            qpp = psa.tile([128, 256], F32, tag="proj")
            nc.tensor.matmul(qpp, W_["Pq"][:, cols], q_sb, start=True, stop=True)
            qp_sb = sb3.tile([128, 256], BF16, tag="qp_sb")
            nc.scalar.copy(qp_sb, qpp)
            scratch = sb3.tile([128, 2, 128], BF16, tag="ttr_scratch")
            nc.vector.tensor_mul(scratch,
                                 qp_sb.rearrange("p (c n) -> p c n", c=2),
                                 k_sb[:, s_, :].rearrange("p (c n) -> p c n", c=2))
            sc = sb3.tile([128, 2, 1], F32, tag="sc")
            nc.vector.tensor_reduce(sc, scratch, axis=AX.X, op=ALU.add)
            nc.scalar.activation(e32[:, s_, :].rearrange("p (c one) -> p c one", c=2),
                                 sc, AF.Exp)
            nc.vector.tensor_copy(e_sb[:, s_, :], e32[:, s_, :])

        # intra-attention softmax (exp grouped with the score exps above)
        paT_all = sb2.tile([S, BPC, 2, S], BF16, tag="paT_all")
        for bl in range(BPC):
            for hh in range(2):
                ex = sb3.tile([S, S], BF16, tag="ex")
                nc.scalar.activation(ex, ms_all[:, bl, hh, :], AF.Exp)
                rs = sb3.tile([S, 1], F32, tag="rs")
                nc.vector.tensor_reduce(rs, ex, axis=AX.X, op=ALU.add)
                ri = sb3.tile([S, 1], F32, tag="ri")
                nc.vector.reciprocal(ri, rs)
                pa = sb3.tile([S, S], BF16, tag="pa")
                nc.vector.tensor_scalar_mul(pa, ex, ri)
                ptp = psb.tile([S, S], BF16, tag="tp", name="ptp")
                nc.tensor.transpose(ptp, pa, W_["id128"][0:S, 0:S])
                nc.vector.tensor_copy(paT_all[:, bl, hh, :], ptp)

        # esum per token + weighted values, co-located in one PSUM bank:
        # esum at cols 256:258, o at cols 0:256; single start=True clears bank.
        acc = psf.tile([128, 512], F32, tag="acc")
        esum_ps = acc[:, 256:258]
        o_ps = acc[:, 0:256]
        acc_insts = []
        for s_ in range(NST):
            cols = slice(s_ * 128, (s_ + 1) * 128)
            acc_insts.append(nc.tensor.matmul(
                esum_ps, W_["Pi"][:, cols], e_sb[:, s_, :],
                start=(s_ == 0), stop=False))
        ow_last = None
        for s_ in range(NST):
            cols = slice(s_ * 128, (s_ + 1) * 128)
            vw = sb3.tile([128, 256], BF16, tag="vw")
            for hh in range(2):
                hs = slice(hh * 128, (hh + 1) * 128)
                nc.vector.tensor_scalar_mul(vw[:, hs], v_sb[:, s_, hs],
                                            e32[:, s_, hh:hh + 1])
            ow_last = nc.tensor.matmul(o_ps, W_["Pi"][:, cols], vw,
                                       start=False, stop=(s_ == NST - 1))
            acc_insts.append(ow_last)
        _coloc(acc_insts)
        einv = sb2.tile([128, 2], F32, tag="einv")
        _after(nc.vector.reciprocal(einv, esum_ps), ow_last)
        o_i = sb2.tile([128, 256], BF16, tag="o_i")
        for hh in range(2):
            hs = slice(hh * 128, (hh + 1) * 128)
            _after(nc.vector.tensor_scalar_mul(o_i[:, hs], o_ps[:, hs],
                                               einv[:, hh:hh + 1]), ow_last)

        if KLEVEL == 27:
            ob = sb2.tile([128, 256], F32, tag="out_sb", name="ob")
            nc.vector.tensor_copy(ob, o_i)
            nc.sync.dma_start(out=d_out.ap(), in_=ob)
            return

        oiT = sb2.tile([128, 2, 128], BF16, tag="oiT")
        for ci in range(2):
            tp = psb.tile([128, 128], BF16, tag="tp", name="tp")
            nc.tensor.transpose(tp, o_i[:, ci * 128:(ci + 1) * 128], W_["id128"])
            nc.vector.tensor_copy(oiT[:, ci, :], tp)

        va_sb = []
        for bl in range(BPC):
            vp = proj([hT_all[:, 0, bl, :], hT_all[:, 1, bl, :], xlast[:, bl, :]],
                      [W_["avw0"], W_["avw1"], W_["avwx"]], W_["avb"], m_parts=S)
            vb = sb2.tile([S, 256], BF16, tag="va_sb")
            nc.scalar.copy(vb, vp)
            va_sb.append(vb)

        oaT = sb2.tile([128, 2, 128], BF16, tag="oaT")
        for bl in range(BPC):
            for hh in range(2):
                op = psb.tile([128, S], F32, tag="tp")
                nc.tensor.matmul(op, va_sb[bl][:, hh * 128:(hh + 1) * 128],
                                 paT_all[:, bl, hh, :], start=True, stop=True)
                nc.vector.tensor_copy(oaT[:, hh, bl * S:(bl + 1) * S], op)

        if KLEVEL == 29:
            ob = sb2.tile([128, 256], F32, tag="out_sb", name="ob")
            nc.vector.tensor_copy(ob[:, 0:128], oaT[:, 0, :])
            nc.vector.tensor_copy(ob[0:64, 128:256], va_sb[0][:, 0:128])
            nc.sync.dma_start(out=d_out.ap(), in_=ob[:, :])
            return

        # fused final projection
        fo = psf.tile([128, 512], F32, tag="acc", name="fo")[:, 0:256]
        nc.tensor.matmul(fo, oiT[:, 0, :], W_["AiT0"], start=True, stop=False)
        nc.tensor.matmul(fo, oiT[:, 1, :], W_["AiT1"], start=False, stop=False)
        nc.tensor.matmul(fo, oaT[:, 0, :], W_["AaT0"], start=False, stop=False)
        nc.tensor.matmul(fo, oaT[:, 1, :], W_["AaT1"], start=False, stop=False)
        nc.tensor.matmul(fo, hown[0], W_["LhT0"], start=False, stop=False)
        nc.tensor.matmul(fo, hown[1], W_["LhT1"], start=False, stop=False)
        nc.tensor.matmul(fo, xp_own, W_["LxT"], start=False, stop=False)
        nc.tensor.matmul(fo, ones, W_["btot"], start=False, stop=True)
        out_sb = sb2.tile([128, 256], F32, tag="out_sb")
        nc.vector.tensor_copy(out_sb, fo)
        nc.sync.dma_start(out=d_out.ap(), in_=out_sb)


def _build(W, MINACT):
    OFF = [0]
    for t in range(L - 1):
        OFF.append(OFF[t] + W[t])
    TOTAL = OFF[-1] + W[-1]

    nc = bacc.Bacc("TRN2", target_bir_lowering=False, debug=False)
    di = {}

    def inp(name, shape, dt=BF16):
        di[name] = nc.dram_tensor(name, list(shape), dt, kind="ExternalInput")

    inp("xinter", [128, TOTAL])
    inp("xintra", [128, B, S])
    inp("xlast", [1, B, S])
    inp("rT", [128, NSEQ])
    inp("indr", [1, TOTAL])
    inp("wihT", [128, 768])
    inp("whh0T", [128, 768])
    inp("whh1T", [128, 768])
    inp("blobB", [128, BLOBW])
    inp("blobC", [1, 256 * len(_BLOBC_NAMES)])
    inp("blobD", [128, 2 * len(_BLOBD_NAMES)], F32)
    inp("id128e", [128, 128])

    d_out = nc.dram_tensor("out", [NTOK, 256], F32, kind="ExternalOutput")

    with tile.TileContext(nc) as tc:
        _emit(nc, tc, di, d_out, W, OFF, MINACT)
    nc.compile()
    return nc


# ----------------------------------------------------------------------------
# host-side prep
# ----------------------------------------------------------------------------

def _plan(inter_len):
    """Per-core length-sort plan + shared compile-time widths."""
    lens5 = np.asarray(inter_len, np.int64).reshape(B, S, R)
    orders, lens_sorted = [], []
    act = np.zeros((NCORES, L), np.int64)
    for c in range(NCORES):
        lens = lens5[[2 * c, 2 * c + 1]].reshape(NSEQ)
        order = np.argsort(-lens, kind="stable")
        ls = lens[order]
        orders.append(order)
        lens_sorted.append(ls)
        for t in range(L):
            act[c, t] = int((ls > t).sum())
    W = [min(NSEQ, int(-32 * (-(act[:, t].max()) // 32))) for t in range(L)]
    W = [max(32, w) for w in W]
    for t in range(1, L):
        W[t] = min(W[t], W[t - 1])
    MINACT = [int(act[:, t].min()) for t in range(L)]
    OFF = [0]
    for t in range(L - 1):
        OFF.append(OFF[t] + W[t])
    return orders, lens_sorted, W, MINACT, OFF


def prep_in_maps(inputs):
    inp = {k: np.asarray(v) for k, v in inputs.items()}
    w_ih = f32c(inp["w_ih"])
    w_hh = f32c(inp["w_hh"])
    b_ih = f32c(inp["b_ih"])
    b_hh = f32c(inp["b_hh"])
    b_rz = b_ih[:2 * H] + b_hh[:2 * H]
    sq = np.sqrt(128.0)

    e = np.exp(f32c(inp["wr"])[0, 0] - f32c(inp["wr"])[0, 0].max())
    w01 = e / e.sum()
    ln_w = f32c(inp["ln_w"])
    L_v, L_h, L_x = ln_w[:, :H], ln_w[:, H:2 * H], ln_w[:, 2 * H:]
    Ai = w01[0] * (L_v @ f32c(inp["io_w"]))
    Aa = w01[1] * (L_v @ f32c(inp["ao_w"]))
    btot = f32c(inp["ln_b"]) + L_v @ (w01[0] * f32c(inp["io_b"]) + w01[1] * f32c(inp["ao_b"]))

    iq_w = f32c(inp["iq_w"]) / sq
    iq_b = f32c(inp["iq_b"]) / sq
    aq_w = f32c(inp["aq_w"]) / sq
    aq_b = f32c(inp["aq_b"]) / sq

    def chunks2(m):  # [128,2] fp32 per-partition chunk tiles
        return f32c(np.stack([m[:128], m[128:256]], axis=1))

    orders, lens_sorted, W, MINACT, OFF = _plan(inp["inter_len"])
    TOTAL = OFF[-1] + W[-1]

    x_bs = f32c(inp["intra_x"])                     # [B,S,D]
    his5 = f32c(inp["inter_his"]).reshape(B, S, R, L, D)
    r5 = f32c(inp["inter_r"]).reshape(B, S, R, D)

    # shared weight blobs
    bw = {
        "iqw0": iq_w.T[0:128], "iqw1": iq_w.T[128:256], "iqwx": iq_w.T[256:383],
        "ikw0": inp["ik_w"].T[0:128], "ikw1": inp["ik_w"].T[128:256],
        "ikwx": inp["ik_w"].T[256:383],
        "ivw0": inp["iv_w"].T[0:128], "ivw1": inp["iv_w"].T[128:256],
        "ivwx": inp["iv_w"].T[256:384],
        "aqw": aq_w.T, "akw": f32c(inp["ak_w"]).T,
        "avw0": inp["av_w"].T[0:128], "avw1": inp["av_w"].T[128:256],
        "AiT0": Ai.T[0:128], "AiT1": Ai.T[128:256],
        "AaT0": Aa.T[0:128], "AaT1": Aa.T[128:256],
        "LhT0": L_h.T[0:128], "LhT1": L_h.T[128:256], "LxT": L_x.T,
        "id128": np.eye(128, dtype=np.float32),
        "cmask": np.where(np.tril(np.ones((S, S), bool)), 0.0, -BIG),
    }
    blobC = np.zeros((1, 256 * len(_BLOBC_NAMES)), np.float32)
    bc = {
        "iqb": iq_b, "ikb": f32c(inp["ik_b"]), "ivb": f32c(inp["iv_b"]),
        "avwx": f32c(inp["av_w"]).T[256], "avb": f32c(inp["av_b"]), "btot": btot,
    }
    for i, nm in enumerate(_BLOBC_NAMES):
        blobC[0, 256 * i:256 * i + len(bc[nm])] = bc[nm]
    blobD = np.zeros((128, 2 * len(_BLOBD_NAMES)), np.float32)
    bd = {
        "b_r": chunks2(b_rz[:H]), "nb_z": chunks2(-b_rz[H:]),
        "b_in": chunks2(b_ih[2 * H:]), "b_hn": chunks2(b_hh[2 * H:]),
        "aqb": chunks2(aq_b), "akb": chunks2(f32c(inp["ak_b"])),
    }
    for i, nm in enumerate(_BLOBD_NAMES):
        blobD[:, 2 * i:2 * i + 2] = bd[nm]

    # negate the z-gate (columns 256:512 of the transposed weights) so the
    # z preactivation accumulates negated and sigmoid needs no scale flip
    wihT_h = w_ih.T.copy()
    wihT_h[:, 256:512] *= -1.0
    whhT_h = w_hh.T.copy()
    whhT_h[:, 256:512] *= -1.0
    shared = dict(
        wihT=bfc(wihT_h),
        whh0T=bfc(whhT_h[0:128]),
        whh1T=bfc(whhT_h[128:256]),
        blobC=bfc(blobC),
        blobD=f32c(blobD),
        id128e=bfc(np.eye(128, dtype=np.float32)),
    )

    in_maps = []
    for c in range(NCORES):
        bsel = [2 * c, 2 * c + 1]
        order = orders[c]
        ls = lens_sorted[c]
        # inter: sorted seq order; orig col order is ((bl,s),r)
        his_cols = his5[bsel].reshape(NSEQ, L, D)[order]    # [NSEQ, L, D]
        xint = np.zeros((D, TOTAL), np.float32)
        ind = np.zeros((1, TOTAL), np.float32)
        for t in range(L):
            o, w = OFF[t], W[t]
            xint[:, o:o + w] = his_cols[:w, t, :].T
            ind[0, o:o + w] = -BIG * (t >= ls[:w])
        rTc = r5[bsel].reshape(NSEQ, D)[order].T            # [D, NSEQ]
        # permutation matrices: tok_of[p] for sorted position p
        tok_of = order // R
        Pq = np.zeros((128, NSEQ), np.float32)
        Pi = np.zeros((128, NSEQ), np.float32)
        for s_ in range(NST):
            for pl in range(128):
                tok = tok_of[s_ * 128 + pl]
                Pq[tok, s_ * 128 + pl] = 1.0
                Pi[pl, s_ * 128 + tok] = 1.0
        blobB = np.zeros((128, BLOBW), np.float32)
        for nm, (o_, w_) in _BLOB_OFF.items():
            src = {"Pq": Pq, "Pi": Pi}.get(nm)
            if src is None:
                src = bw[nm]
            blobB[0:src.shape[0], o_:o_ + src.shape[1]] = src
        # intra: batches rotated so own batches are 0..1; (d, b, s) layout
        rolled = np.roll(x_bs, -2 * c, axis=0)
        xia = rolled.transpose(2, 0, 1)             # [D, B, S]
        m = dict(shared)
        m.update(
            xinter=bfc(xint),
            xintra=bfc(xia),
            xlast=bfc(xia[127:128]),
            rT=bfc(rTc),
            indr=bfc(ind),
            blobB=bfc(blobB),
        )
        in_maps.append(m)
    return in_maps, W, MINACT


def assemble(core_outs):
    o = np.stack([np.asarray(co, np.float32) for co in core_outs])  # [8,128,256]
    return np.ascontiguousarray(o.reshape(B * S, 256))


_CACHE = {}


def kernel(**inputs) -> np.ndarray:
    in_maps, W, MINACT = prep_in_maps(inputs)
    key = (tuple(W), tuple(MINACT))
    if _CACHE.get("key") != key:
        _CACHE["nc"] = _build(W, MINACT)
        _CACHE["key"] = key
    nc = _CACHE["nc"]
    res = run_bass_kernel_spmd(nc, in_maps, core_ids=list(range(NCORES)))
    return assemble([r["out"] for r in res.results])
